# revision 1
# baseline (speedup 1.0000x reference)
"""DopplerPTNet point-transformer block on 8 Trainium2 NeuronCores — v2.

Strategy (point-parallel, replicated k/v table build):
  - Each core owns N/8 query points.  Every core builds the FULL [N, 388]
    bf16 "kv table" locally from replicated transposed features (no
    collective): one row per point = [ k(256 fp8e4m3, pre-scaled by bn1
    gamma/rsqrt(var)) | v(256 bf16) | tg(3 bf16) ], tg = A1@xyz + c1 being
    the position encoder's first affine.  A separate [N] bf16 vh table
    holds the velocity encoding scalar (post BN+ReLU).
  - Main pass per 128-point tile (4-tile chunks): ONE merged indirect DMA
    gathers 64 x 16 kv rows point-major into SBUF; a second indirect DMA
    gathers vh scalars straight into the geometry tile.  The attention
    logit MLP runs channel-major: k reaches channel-major via PE-transpose
    matmuls accumulated directly into the logit PSUM (no SBUF transpose
    pass), the position/velocity encoding enters via K=5 matmuls against a
    transposed geometry tile whose 5th row adds the folded BN shift, and q
    is subtracted via an identity matmul of the pre-negated q.  Softmax and
    the share-grouped aggregation run point-major on DVE/Pool; the position
    encoding part of the aggregated values is reconstructed from the
    attention-weighted geometry moments (U-trick) instead of materializing
    [N,NS,C] values.

All BatchNorms are inference-affine and folded on the host.  w_b2 is
dropped (softmax-invariant); bk-bq+p_b2+v_b2 folds into shift1; bv+p_b2+
v_b2 folds into r_bn's mean because sum(attn)==1.
"""

import sys

sys.path.insert(0, "/opt/trn_rl_repo")

import numpy as np
import ml_dtypes

import concourse.bass as bass
import concourse.mybir as mybir
import concourse.tile as tile
from concourse import bacc
from concourse.bass import IndirectOffsetOnAxis
from concourse.masks import make_identity

BF16 = mybir.dt.bfloat16
F32 = mybir.dt.float32
F8 = mybir.dt.float8e4
I32 = mybir.dt.int32
AOP = mybir.AluOpType
AFT = mybir.ActivationFunctionType

NCORES = 8
C = 256
NS = 16
CS = 32
S = 8
P = 128
ROW = 388          # bf16 elems per kv row: k 128(=256 fp8) | v 256 | tg 3 | pad
EPS = 1e-5
CH = 2             # tiles per phase-D chunk
BCH = 2            # tiles per phase-B chunk


def _bf(x):
    return np.ascontiguousarray(
        np.asarray(x, dtype=np.float32).astype(ml_dtypes.bfloat16))


def _f32(x):
    return np.ascontiguousarray(x, dtype=np.float32)


def build_program(n_total: int, use_collective: bool = False, phases: str = "all", taps: bool = False):
    npc = n_total // NCORES
    nt = npc // P                 # phase-D tiles per core
    nb = npc if use_collective else n_total
    ntb = nb // P                 # phase-B tiles per core

    nc = bacc.Bacc(
        "TRN2",
        target_bir_lowering=False,
        debug=False,
        enable_asserts=False,
        num_devices=NCORES,
    )

    # ---- I/O -----------------------------------------------------------
    def inp(name, shape, dt):
        return nc.dram_tensor(name, shape, dt, kind="ExternalInput")

    # phase-B inputs (full N if replicated, own shard if collective)
    featsT_b = inp("featsT_b", [2, P, nb], BF16)
    xyz4_b = inp("xyz4_b", [nb, 4], F32)
    vel_b = inp("vel_b", [P, nb // P], F32)
    # phase-D per-core inputs
    featsT_own = inp("featsT_own", [2, P, npc], BF16)
    featsb_sh = inp("featsb_sh", [npc, C], BF16)
    xyz4_sh = inp("xyz4_sh", [npc, 4], F32)
    idx_sh = inp("idx_sh", [npc, NS], I32)
    idxv_sh = inp("idxv_sh", [npc, NS], I32)
    idxvf_sh = inp("idxvf_sh", [npc // P, P, NS * P // 16], mybir.dt.int16)
    voh_sh = inp("voh_sh", [npc, NS * 64], BF16)   # one-hot of idx_v % 64
    # folded weights
    wqn_t = inp("wqn_t", [2, P, C], BF16)      # -(scale1*Wq).T cin-groups
    wkv_t = inp("wkv_t", [2, P, 2 * C], BF16)  # [(scale1*Wk).T | Wv.T]
    w2w5 = inp("w2w5", [32, 4, 2, P], BF16)    # 8-banded [pw2 s1;vw2 s1;sh1]
    w2v_r = inp("w2v_r", [1, 4 * C], BF16)     # pe rhs rows (unscaled)
    w1_t = inp("w1_t", [2, P, CS], BF16)       # (scale2*w_w1).T
    w2_t = inp("w2_t", [CS, CS], BF16)         # w_w2.T
    rw2 = inp("rw2", [2, P, C], BF16)          # r_w.T
    a1_t = inp("a1_t", [64, 8, 4], F32)        # A1.T 8-banded
    c1_c = inp("c1_c", [1, 4], F32)            # pos affine bias
    svbv_c = inp("svbv_c", [1, 2], F32)        # velocity scale/bias
    sh2_c = inp("sh2_c", [CS, 1], F32)         # bn2 shift per cs
    sclr_c = inp("sclr_c", [P, 2], F32)        # rho bn scale (ch-major)
    shfr_c = inp("shfr_c", [P, 2], F32)        # rho bn shift (ch-major)
    rb_c = inp("rb_c", [1, C], F32)            # r_b

    out_ext = nc.dram_tensor("out", [npc, C], F32, kind="ExternalOutput")
    tap_t = {}
    if taps:
        for nm, shp, dt in [
            ("tap_stag", [P, NS * ROW], BF16),
            ("tap_pmx", [P, NS * 8], BF16),
            ("tap_T2", [32, 4 * P], BF16),
            ("tap_qn", [P, 2 * P], BF16),
            ("tap_t1", [P, 2 * NS * P], BF16),
            ("tap_t2o", [CS, NS * P], BF16),
            ("tap_attnE", [P, NS * CS], BF16),
            ("tap_rin", [P, C], BF16),
            ("tap_kv", [P, 4 * ROW], BF16),
            ("tap_vh", [P, 16], BF16),
        ]:
            tap_t[nm] = nc.dram_tensor(nm, shp, dt, kind="ExternalOutput")

    # ---- internal DRAM -------------------------------------------------
    if use_collective:
        kv_shard = nc.dram_tensor("kv_shard", [npc, ROW], BF16)
        vh_shard = nc.dram_tensor("vh_shard", [npc, 1], F32)
        kv = nc.dram_tensor("kv_full", [n_total, ROW], BF16,
                            addr_space="Shared")
        vh = nc.dram_tensor("vh_full", [n_total, 1], F32,
                            addr_space="Shared")
    else:
        kv_shard, vh_shard = None, None
        kv = nc.dram_tensor("kv_full", [n_total, ROW], BF16)
        vh = nc.dram_tensor("vh_full", [n_total, 1], F32)

    rg = [list(range(NCORES))]

    with tile.TileContext(nc) as tc:
        with (
            tc.tile_pool(name="const", bufs=1) as cpool,
            tc.tile_pool(name="work", bufs=2) as pool,
            tc.tile_pool(name="big", bufs=2) as bigpool,
            tc.tile_pool(name="ps_t1", bufs=2, space="PSUM") as ps_t1,
            tc.tile_pool(name="ps_t2", bufs=2, space="PSUM") as ps_t2,
            tc.tile_pool(name="ps_e", bufs=2, space="PSUM") as ps_e,
            tc.tile_pool(name="ps_l", bufs=2, space="PSUM") as ps_l,
        ):
            # ---------- constants ----------
            ident_b = cpool.tile([P, P], BF16, tag="ident_b")
            make_identity(nc, ident_b[:])
            ident_f = cpool.tile([P, P], F32, tag="ident_f")
            make_identity(nc, ident_f[:])

            def cload(src, shape, dt, tag):
                t = cpool.tile(shape, dt, tag=tag)
                nc.sync.dma_start(out=t[:], in_=src)
                return t

            def gload(src, width, tag):
                t = cpool.tile([P, 2, width], BF16, tag=tag)
                for g in range(2):
                    nc.sync.dma_start(out=t[:, g, :], in_=src[g, :, :])
                return t

            wqn_sb = gload(wqn_t, C, "wqn")
            wkv_sb = gload(wkv_t, 2 * C, "wkv")
            w1_sb = gload(w1_t, CS, "w1")
            rw2_sb = gload(rw2, C, "rw2")
            w2w5_sb = cload(w2w5[:, :, :, :], [32, 4, 2, P], BF16, "w2w5")
            w2_sb = cload(w2_t[:, :], [CS, CS], BF16, "w2")
            a1_sb = cload(a1_t[:, :, :], [64, 8, 4], F32, "a1")
            sh2_sb = cload(sh2_c[:, :], [CS, 1], F32, "sh2")
            sclr_sb = cload(sclr_c[:, :], [P, 2], F32, "sclr")
            shfr_sb = cload(shfr_c[:, :], [P, 2], F32, "shfr")

            def bcast(name, src, width, dt):
                row = cpool.tile([1, width], dt, tag=name + "r")
                nc.sync.dma_start(out=row[:], in_=src)
                full = cpool.tile([P, width], dt, tag=name)
                nc.gpsimd.partition_broadcast(full[:], row[:])
                return full

            svbv_sb = bcast("svbv", svbv_c[:, :], 2, F32)
            c1_sb = bcast("c1", c1_c[:, :], 4, F32)
            w2vR = bcast("w2vR", w2v_r[:, :], 4 * C, BF16)
            rb_sb = bcast("rb", rb_c[:, :], C, F32)

            # ---------- phase V: velocity table (bulk) ----------
            nvb = (nb // P) if phases in ("all", "VB") else 0
            for lo in range(0, nvb, 256):
                w = min(256, nvb - lo)
                vel_t = bigpool.tile([P, w], F32, tag="vel_t")
                nc.sync.dma_start(out=vel_t[:], in_=vel_b[:, lo:lo + w])
                vhb = bigpool.tile([P, w], F32, tag="vhb")
                nc.scalar.activation(
                    vhb[:], vel_t[:], AFT.Relu,
                    bias=svbv_sb[:, 1:2], scale=svbv_sb[:, 0:1],
                )
                dst = vh_shard if use_collective else vh
                nc.sync.dma_start(
                    out=dst[:, :]
                    .rearrange("(p j) o -> p (j o)", p=P)[:, lo:lo + w],
                    in_=vhb[:],
                )

            # ---------- phase B: kv table build (skewed pipeline) ----------
            kv_dst = kv_shard if use_collective else kv
            XB = 8                    # xyz tiles per transpose batch
            if phases == "D":
                ntb = 0

            def b_loads(cc):
                ld = {"cc": cc}
                if cc % XB == 0:
                    xw = min(XB, ntb - cc)
                    xyz_t = pool.tile([P, XB, 8], F32, tag="xyz_b")
                    nc.vector.memset(xyz_t[:, :, 4:8], 0)
                    if xw < XB:
                        nc.vector.memset(xyz_t[:, xw:, 0:4], 0)
                    nc.sync.dma_start(
                        out=xyz_t[:, :xw, 0:4],
                        in_=xyz4_b[cc * P:(cc + xw) * P, :]
                        .rearrange("(j p) d -> p j d", p=P),
                    )
                    ld["xyz_t"] = xyz_t
                    ld["xw"] = xw
                ftT = pool.tile([P, 2, BCH * P], BF16, tag="ftT_b")
                nc.sync.dma_start(
                    out=ftT[:],
                    in_=featsT_b[:, :, cc * P:(cc + BCH) * P]
                    .rearrange("g p n -> p g n"),
                )
                ld["ftT"] = ftT
                return ld

            batch_state = {}

            def b_compute(ld):
                cc = ld["cc"]
                if "xyz_t" in ld:
                    xw = ld["xw"]
                    xT_ps = ps_e.tile([8 * XB, P], F32, tag="ps_e")
                    nc.tensor.transpose(
                        out=xT_ps[:],
                        in_=ld["xyz_t"][:, :, :]
                        .rearrange("p j d -> p (j d)"),
                        identity=ident_f[:],
                    )
                    xT_sb = pool.tile([8 * XB, P], F32, tag="xT_b")
                    nc.scalar.copy(xT_sb[:], xT_ps[:])
                    tg_ps = ps_e.tile([P, XB, 4], F32, tag="ps_e")
                    for j in range(xw):
                        nc.tensor.matmul(
                            out=tg_ps[:, j, :],
                            lhsT=xT_sb[:, :],
                            rhs=a1_sb[:, j, :],
                            start=True, stop=True,
                        )
                    tg_sb = pool.tile([P, XB, 4], BF16, tag="tg_b")
                    nc.vector.tensor_tensor(
        out=tg_sb[:, :xw, :],
        in0=tg_ps[:, :xw, :],
        in1=c1_sb[:, :4]
                        .rearrange("p (o d) -> p o d", o=1)
                        .to_broadcast([P, xw, 4]),
        op=AOP.add)
                    batch_state["tg_sb"] = tg_sb
                    batch_state["c0"] = cc
                ftT = ld["ftT"]
                base = cc * P
                row_t = pool.tile([P, BCH, ROW], BF16, tag="row_t")
                row_f8 = row_t[:, :, 0:P].bitcast(F8)
                for j in range(BCH):
                    kv_ps = ps_t1.tile([P, 2 * C], F32, tag="ps_t1")
                    for g in range(2):
                        nc.tensor.matmul(
                            out=kv_ps[:],
                            lhsT=ftT[:, g, j * P:(j + 1) * P],
                            rhs=wkv_sb[:, g, :],
                            start=(g == 0), stop=(g == 1),
                        )
                    nc.scalar.copy(row_f8[:, j, :], kv_ps[:, 0:C])
                    nc.vector.tensor_copy(
                        row_t[:, j, P:P + C], kv_ps[:, C:2 * C])
                co = cc - batch_state["c0"]
                nc.vector.tensor_copy(
                    row_t[:, :, P + C:P + C + 4],
                    batch_state["tg_sb"][:, co:co + BCH, :],
                )
                nc.sync.dma_start(
                    out=kv_dst[base:base + BCH * P, :]
                    .rearrange("(j p) r -> p j r", p=P),
                    in_=row_t[:],
                )

            prev = None
            for cc in range(0, ntb, BCH):
                cur = b_loads(cc)
                if prev is not None:
                    b_compute(prev)
                prev = cur
            if prev is not None:
                b_compute(prev)

            # ---------- phase C: all-gather tables ----------
            if use_collective:
                nc.gpsimd.collective_compute(
                    "AllGather", AOP.bypass, replica_groups=rg,
                    ins=[kv_shard.ap().opt()], outs=[kv.ap().opt()],
                )
                nc.gpsimd.collective_compute(
                    "AllGather", AOP.bypass, replica_groups=rg,
                    ins=[vh_shard.ap().opt()], outs=[vh.ap().opt()],
                )

            # ---------- phase D: main pass (skewed CH-tile chunks) ------
            if taps:
                kvt = pool.tile([P, 4, ROW], BF16, tag="kvt")
                nc.sync.dma_start(
                    out=kvt[:],
                    in_=kv[0:4 * P, :].rearrange("(j p) r -> p j r", p=P))
                nc.sync.dma_start(
                    out=tap_t["tap_kv"][:, :],
                    in_=kvt[:].rearrange("p j r -> p (j r)"))
                vht = pool.tile([P, 16], BF16, tag="vht")
                nc.sync.dma_start(
                    out=vht[:],
                    in_=vh[0:2048, :].rearrange("(p j) o -> p (j o)", p=P))
                nc.sync.dma_start(out=tap_t["tap_vh"][:, :], in_=vht[:])
            if phases == "VB":
                nt = 0

            def d_loads(tc0):
                cw = min(CH, nt - tc0)
                rsl = slice(tc0 * P, (tc0 + cw) * P)
                ld = {"tc0": tc0, "cw": cw}
                idx_t = pool.tile([P, CH, NS], I32, tag="idx_t")
                nc.sync.dma_start(
                    out=idx_t[:, :cw, :],
                    in_=idx_sh[rsl, :].rearrange("(j p) s -> p j s", p=P),
                )
                idxv_t = pool.tile([P, CH, NS], I32, tag="idxv_t")
                nc.sync.dma_start(
                    out=idxv_t[:, :cw, :],
                    in_=idxv_sh[rsl, :].rearrange("(j p) s -> p j s", p=P),
                )
                xyz_t = pool.tile([P, CH, 8], F32, tag="xyz_t")
                nc.vector.memset(xyz_t[:, :, 4:8], 0)
                nc.sync.dma_start(
                    out=xyz_t[:, :cw, 0:4],
                    in_=xyz4_sh[rsl, :].rearrange("(j p) d -> p j d", p=P),
                )
                fpm = pool.tile([P, CH, C], BF16, tag="fpm")
                nc.sync.dma_start(
                    out=fpm[:, :cw, :],
                    in_=featsb_sh[rsl, :].rearrange("(j p) c -> p j c", p=P),
                )
                ftT4 = pool.tile([P, 2, CH * P], BF16, tag="ftT4")
                nc.sync.dma_start(
                    out=ftT4[:, :, : cw * P],
                    in_=featsT_own[:, :, rsl].rearrange("g p n -> p g n"),
                )
                stag = bigpool.tile([P, CH, NS, ROW], BF16, tag="stag")
                pmx = pool.tile([P, CH, NS, 8], BF16, tag="pmx")
                nc.vector.memset(pmx[:, :, :, 4:5], 1.0)
                nc.vector.memset(pmx[:, :, :, 5:8], 0)
                for jj in range(cw):
                    for s in range(NS):
                        nc.gpsimd.indirect_dma_start(
                            out=stag[:, jj, s, :],
                            out_offset=None,
                            in_=kv[:, :],
                            in_offset=IndirectOffsetOnAxis(
                                ap=idx_t[:, jj, s:s + 1], axis=0),
                        )
                idxvf_t = pool.tile([P, CH, NS * P // 16], mybir.dt.int16,
                                    tag="idxvf_t")
                nc.sync.dma_start(
                    out=idxvf_t[:, :cw, :],
                    in_=idxvf_sh[tc0:tc0 + cw, :, :]
                    .rearrange("j p w -> p j w"),
                )
                velf = bigpool.tile([P, CH, NS, 64], F32, tag="velf")
                voh = bigpool.tile([P, CH, NS * 64], BF16, tag="voh")
                nc.sync.dma_start(
                    out=voh[:, :cw, :],
                    in_=voh_sh[rsl, :].rearrange("(j p) w -> p j w", p=P),
                )
                nc.gpsimd.dma_gather(
                    out_ap=velf[:, :cw, :, :]
                    .rearrange("p j s k -> p (j s) k"),
                    in_ap=vh[:, :].rearrange("(m k) o -> m (k o)", k=64),
                    idxs_ap=idxvf_t[:, :cw, :]
                    .rearrange("p j w -> p (j w)"),
                    num_idxs=cw * NS * P,
                    num_idxs_reg=cw * NS * P,
                    elem_size=64,
                    transpose=False,
                    single_packet=False,
                )
                ld_extra = {"velf": velf, "voh": voh}
                ld.update(xyz_t=xyz_t, fpm=fpm, ftT4=ftT4, stag=stag,
                          pmx=pmx, **ld_extra)
                return ld

            def d_compute(ld):
                tc0, cw = ld["tc0"], ld["cw"]
                xyz_t, fpm = ld["xyz_t"], ld["fpm"]
                ftT4, stag, pmx = ld["ftT4"], ld["stag"], ld["pmx"]
                base = tc0 * P
                xa_ps = ps_e.tile([P, 128], F32, tag="ps_e")
                xT_ps = xa_ps[0:CH * 8, 0:P]
                nc.tensor.transpose(
                    out=xT_ps,
                    in_=xyz_t[:, :, :].rearrange("p j d -> p (j d)"),
                    identity=ident_f[:],
                )
                xT_sb = pool.tile([CH * 8, P], F32, tag="xT_sb")
                nc.scalar.copy(xT_sb[:], xT_ps)
                axc_ps = ps_e.tile([P, CH, 4], F32, tag="ps_e")
                for j in range(cw):
                    nc.tensor.matmul(
                        out=axc_ps[:, j, :3],
                        lhsT=xT_sb[:, :],
                        rhs=a1_sb[0:CH * 8, j, :3],
                        start=True, stop=True,
                    )
                nc.vector.tensor_tensor(
                    out=pmx[:, :cw, :, 0:3],
                    in0=stag[:, :cw, :, P + C:P + C + 3],
                    in1=axc_ps[:, :cw, :3]
                    .rearrange("p j (o d) -> p j o d", o=1)
                    .to_broadcast([P, cw, NS, 3]),
                    op=AOP.subtract,
                )
                nc.vector.tensor_scalar_max(
                    pmx[:, :cw, :, 0:3], pmx[:, :cw, :, 0:3], 0.0)
                velf, voh = ld["velf"], ld["voh"]
                vprod = bigpool.tile([P, CH, NS, 64], BF16, tag="vprod")
                nc.vector.tensor_tensor(
                    out=vprod[:, :cw, :, :],
                    in0=velf[:, :cw, :, :],
                    in1=voh[:, :cw, :].rearrange("p j (s k) -> p j s k",
                                                 k=64),
                    op=AOP.mult,
                )
                with nc.allow_low_precision(reason="one-hot select sum"):
                    nc.vector.tensor_reduce(
                        out=pmx[:, :cw, :, 3:4],
                        in_=vprod[:, :cw, :, :],
                        axis=mybir.AxisListType.X,
                        op=AOP.add,
                    )
                for j in range(cw):
                    _tile_body(nc, ps_t1, ps_t2, ps_e, ps_l, pool, bigpool,
                               ident_b, wqn_sb, w2w5_sb, w1_sb, w2_sb,
                               sh2_sb, w2vR, sclr_sb, shfr_sb, rb_sb,
                               rw2_sb, stag, pmx, ftT4, fpm, out_ext,
                               base, j,
                               tap_t if (tc0 == 0 and j == 0) else None)

            prev = None
            for tc0 in range(0, nt, CH):
                cur = d_loads(tc0)
                if prev is not None:
                    d_compute(prev)
                prev = cur
            if prev is not None:
                d_compute(prev)

    nc.compile()
    return nc


def _tile_body(nc, ps_t1, ps_t2, ps_e, ps_l, pool, bigpool, ident_b, wqn_sb,
               w2w5_sb, w1_sb, w2_sb, sh2_sb, w2vR, sclr_sb, shfr_sb,
               rb_sb, rw2_sb, stag, pmx, ftT4, fpm, out_ext, base, j,
               tap_t=None):
    def tap(nm, ap):
        if tap_t is not None and nm in tap_t:
            nc.sync.dma_start(out=tap_t[nm][:, :], in_=ap)
    """Compute one 128-point tile (index j within the current chunk)."""
    stag_f8 = stag[:, j, :, 0:P].bitcast(F8)        # [P, NS, 256] fp8 k
    v_view = stag[:, j, :, P:P + C]                 # [P, NS, 256] bf16 v

    # ---- q (negated, channel-major) ----
    q_ps = ps_e.tile([P, 2, P], F32, tag="ps_e")
    for cg in range(2):
        for g in range(2):
            nc.tensor.matmul(
                out=q_ps[:, cg, :],
                lhsT=wqn_sb[:, g, cg * P:(cg + 1) * P],
                rhs=ftT4[:, g, j * P:(j + 1) * P],
                start=(g == 0), stop=(g == 1),
            )
    qn_sb = pool.tile([P, 2, P], BF16, tag="qn_sb")
    nc.scalar.copy(qn_sb[:], q_ps[:])
    tap("tap_stag", stag[:, j, :, :].rearrange("p s r -> p (s r)"))
    tap("tap_pmx", pmx[:, j, :, :].rearrange("p s d -> p (s d)"))
    tap("tap_qn", qn_sb[:, :, :].rearrange("p g n -> p (g n)"))
    q_rep = pool.tile([P, 2, 4 * P], BF16, tag="q_rep")
    for cg in range(2):
        nc.vector.tensor_copy(
            q_rep[:, cg, :],
            qn_sb[:, cg, :].rearrange("p (o n) -> p o n", o=1)
            .to_broadcast([P, 4, P]),
        )

    # ---- geometry transpose: pmx[j] -> T2 [4 groups][8u+d, pt] ----
    t2g_ps = ps_e.tile([32, 4, P], BF16, tag="ps_e")
    for g in range(4):
        nc.tensor.transpose(
            out=t2g_ps[:, g, :],
            in_=pmx[:, j, 4 * g:4 * g + 4, :].rearrange("p s d -> p (s d)"),
            identity=ident_b[:],
        )
    T2 = pool.tile([32, 4, P], BF16, tag="T2")
    nc.scalar.copy(T2[:], t2g_ps[:])
    tap("tap_T2", T2[:, :, :].rearrange("k g n -> k (g n)"))

    # ---- t1 = relu(k' - q' + pe' + shift1), channel-major ----
    t1 = bigpool.tile([P, 2, NS * P], BF16, tag="t1")
    for cg in range(2):
        for h in range(4):
            wps = ps_t1.tile([P, 4 * P], F32, tag="ps_t1")
            nc.tensor.matmul(
                out=wps[:],
                lhsT=ident_b[:],
                rhs=q_rep[:, cg, :],
                start=True, stop=False,
                skip_group_check=True,
            )
            for s4 in range(4):
                s = h * 4 + s4
                nc.tensor.matmul(
                    out=wps[:, s4 * P:(s4 + 1) * P],
                    lhsT=w2w5_sb[:, s % 4, cg, :],
                    rhs=T2[:, s // 4, :],
                    start=False, stop=False,
                    skip_group_check=True,
                )
                nc.tensor.matmul(
                    out=wps[:, s4 * P:(s4 + 1) * P],
                    lhsT=stag_f8[:, s, cg * P:(cg + 1) * P],
                    rhs=ident_b[:],
                    start=False, stop=(s4 == 3),
                    skip_group_check=True,
                )
            nc.scalar.activation(
                t1[:, cg, h * 4 * P:(h + 1) * 4 * P], wps[:], AFT.Relu)

    tap("tap_t1", t1[:, :, :].rearrange("p g n -> p (g n)"))
    # ---- W1 (256->32) + bn2 + relu ----
    t2 = pool.tile([CS, NS * P], BF16, tag="t2")
    for q4 in range(4):
        w1_ps = ps_t2.tile([CS, 4 * P], F32, tag="ps_t2")
        for cg in range(2):
            nc.tensor.matmul(
                out=w1_ps[:],
                lhsT=w1_sb[:, cg, :],
                rhs=t1[:, cg, q4 * 4 * P:(q4 + 1) * 4 * P],
                start=(cg == 0), stop=(cg == 1),
            )
        nc.scalar.activation(
            t2[:, q4 * 4 * P:(q4 + 1) * 4 * P], w1_ps[:],
            AFT.Relu, bias=sh2_sb[:, 0:1], scale=1.0,
        )

    tap("tap_t2o", t2[:, :])
    # ---- W2 -> point-major logits, exp ----
    attn_ps = ps_t2.tile([P, NS * CS], F32, tag="ps_t2")  # shares t2 slots
    for s in range(NS):
        nc.tensor.matmul(
            out=attn_ps[:, s * CS:(s + 1) * CS],
            lhsT=t2[:, s * P:(s + 1) * P],
            rhs=w2_sb[:, :],
            start=True, stop=True,
        )
    attnE = pool.tile([P, NS * CS], BF16, tag="attnE")
    nc.scalar.activation(attnE[:], attn_ps[:], AFT.Exp)

    tap("tap_attnE", attnE[:, :])
    # ---- softmax over ns ----
    scr = pool.tile([P, 12 * CS], BF16, tag="scr")
    v0 = attnE[:, :].rearrange("p (s c) -> p s c", c=CS)
    r1 = scr[:, 0:8 * CS].rearrange("p (s c) -> p s c", c=CS)
    nc.vector.tensor_tensor(
        out=r1, in0=v0[:, 0:16:2, :], in1=v0[:, 1:16:2, :],
        op=AOP.add)
    r2 = scr[:, 8 * CS:12 * CS].rearrange("p (s c) -> p s c", c=CS)
    nc.vector.tensor_tensor(
        out=r2,
        in0=r1[:, 0:8:2, :],
        in1=r1[:, 1:8:2, :],
        op=AOP.add)
    ssum = pool.tile([P, 3 * CS], F32, tag="ssum")
    s3 = ssum[:, CS:3 * CS].rearrange("p (s c) -> p s c", c=CS)
    nc.vector.tensor_tensor(
        out=s3,
        in0=r2[:, 0:4:2, :],
        in1=r2[:, 1:4:2, :],
        op=AOP.add)
    nc.vector.tensor_tensor(
        out=ssum[:, 0:CS].rearrange("p (o c) -> p o c", o=1),
        in0=s3[:, 0:1, :],
        in1=s3[:, 1:2, :],
        op=AOP.add)
    rcp = pool.tile([P, 2 * CS], F32, tag="rcp")
    nc.vector.reciprocal(rcp[:, 0:CS], ssum[:, 0:CS])
    rcpb = pool.tile([P, CS], BF16, tag="rcpb")
    nc.vector.tensor_copy(rcpb[:], rcp[:, 0:CS])
    attn_n = pool.tile([P, NS * CS], BF16, tag="attn_n")
    nc.vector.tensor_tensor(
        out=attn_n[:].rearrange("p (s c) -> p s c", c=CS),
        in0=attnE[:].rearrange("p (s c) -> p s c", c=CS),
        in1=rcpb[:].rearrange("p (o c) -> p o c", o=1)
        .to_broadcast([P, NS, CS]),
        op=AOP.mult)

    # ---- U-trick: attention-weighted geometry moments ----
    # Umul[p, s, cs, d] = attn_n[p,s,cs] * pmx[p,j,s,d]   (d = 0..3)
    um = bigpool.tile([P, NS, CS, 4], BF16, tag="um")
    for d in range(4):
        eng = nc.vector
        eng.tensor_tensor(
            out=um[:, :, :, d:d + 1],
            in0=attn_n[:].rearrange("p (s c o) -> p s c o", c=CS, o=1),
            in1=pmx[:, j, :, d:d + 1]
            .rearrange("p s (o d) -> p s o d", o=1)
            .to_broadcast([P, NS, CS, 1]),
            op=AOP.mult,
        )
    # tree-reduce over ns (views of packed 128-wide rows)
    umv = um[:, :, :, :].rearrange("p s c d -> p s (c d)")
    W4 = CS * 4
    ut = pool.tile([P, 15 * W4], BF16, tag="ut")
    u1 = ut[:, 0:8 * W4].rearrange("p (s w) -> p s w", w=W4)
    nc.vector.tensor_tensor(
        out=u1,
        in0=umv[:, 0:16:2, :],
        in1=umv[:, 1:16:2, :],
        op=AOP.add)
    u2 = ut[:, 8 * W4:12 * W4].rearrange("p (s w) -> p s w", w=W4)
    nc.vector.tensor_tensor(
        out=u2,
        in0=u1[:, 0:8:2, :],
        in1=u1[:, 1:8:2, :],
        op=AOP.add)
    u3 = ut[:, 12 * W4:14 * W4].rearrange("p (s w) -> p s w", w=W4)
    nc.vector.tensor_tensor(
        out=u3,
        in0=u2[:, 0:4:2, :],
        in1=u2[:, 1:4:2, :],
        op=AOP.add)
    u3f = ut[:, 14 * W4:15 * W4].rearrange("p (o w) -> p o w", o=1)
    nc.vector.tensor_tensor(
        out=u3f,
        in0=u3[:, 0:2:2, :],
        in1=u3[:, 1:2:2, :],
        op=AOP.add)
    # P4[p, g, cs, d] = U3[p, cs, d] * w2vR[p, (d-major rows)]
    p4 = pool.tile([P, S, CS, 4], BF16, tag="p4")
    nc.vector.tensor_tensor(
        out=p4[:],
        in0=ut[:, 14 * W4:15 * W4]
        .rearrange("p (o c d) -> p o c d", o=1, d=4)
        .to_broadcast([P, S, CS, 4]),
        in1=w2vR[:, :].rearrange("p (g c d) -> p g c d", g=S, d=4),
        op=AOP.mult)
    agg_pe = pool.tile([P, C], F32, tag="agg_pe")
    nc.vector.tensor_reduce(
        out=agg_pe[:].rearrange("p (g c o) -> p g c o", g=S, o=1),
        in_=p4[:],
        axis=mybir.AxisListType.X,
        op=AOP.add,
    )

    # ---- v aggregation: prod over share groups + ns tree ----
    prod = bigpool.tile([P, NS * C], BF16, tag="prod")
    nc.vector.tensor_tensor(
        out=prod[:].rearrange("p (s g c) -> p s g c", g=S, c=CS),
        in0=v_view.rearrange("p s (g c) -> p s g c", c=CS),
        in1=attn_n[:].rearrange("p (s o c) -> p s o c", o=1, c=CS)
        .to_broadcast([P, NS, S, CS]),
        op=AOP.mult)
    tscr = bigpool.tile([P, 14 * C], BF16, tag="tscr")
    pv = prod[:, :].rearrange("p (s c) -> p s c", c=C)
    u1v = tscr[:, 0:8 * C].rearrange("p (s c) -> p s c", c=C)
    nc.vector.tensor_tensor(
        out=u1v,
        in0=pv[:, 0:16:2, :],
        in1=pv[:, 1:16:2, :],
        op=AOP.add)
    u2v = tscr[:, 8 * C:12 * C].rearrange("p (s c) -> p s c", c=C)
    nc.vector.tensor_tensor(
        out=u2v,
        in0=u1v[:, 0:8:2, :],
        in1=u1v[:, 1:8:2, :],
        op=AOP.add)
    u3v = tscr[:, 12 * C:14 * C].rearrange("p (s c) -> p s c", c=C)
    nc.vector.tensor_tensor(
        out=u3v,
        in0=u2v[:, 0:4:2, :],
        in1=u2v[:, 1:4:2, :],
        op=AOP.add)
    aggv = pool.tile([P, C], F32, tag="aggv")
    nc.vector.tensor_tensor(
        out=aggv[:].rearrange("p (o c) -> p o c", o=1),
        in0=u3v[:, 0:1, :],
        in1=u3v[:, 1:2, :],
        op=AOP.add)

    # ---- residual (BN+relu folded into the rho activation) ----
    rin = pool.tile([P, C], BF16, tag="rin")
    nc.vector.tensor_tensor(
        out=rin[:],
        in0=aggv[:],
        in1=agg_pe[:],
        op=AOP.add)
    nc.vector.tensor_tensor(
        out=rin[:],
        in0=rin[:],
        in1=fpm[:, j, :],
        op=AOP.add)
    tap("tap_rin", rin[:])
    rT_ps = ps_l.tile([P, 2, P], BF16, tag="ps_l")
    for cg in range(2):
        nc.tensor.transpose(
            out=rT_ps[:, cg, :],
            in_=rin[:, cg * P:(cg + 1) * P],
            identity=ident_b[:],
        )
    rT_sb = pool.tile([P, 2, P], BF16, tag="rT_sb")
    for cg in range(2):
        nc.scalar.activation(
            rT_sb[:, cg, :], rT_ps[:, cg, :], AFT.Relu,
            bias=shfr_sb[:, cg:cg + 1], scale=sclr_sb[:, cg:cg + 1],
        )
    o_ps = ps_l.tile([P, C], F32, tag="ps_l")
    for cg in range(2):
        nc.tensor.matmul(
            out=o_ps[:],
            lhsT=rT_sb[:, cg, :],
            rhs=rw2_sb[:, cg, :],
            start=(cg == 0), stop=(cg == 1),
        )
    out_sb = pool.tile([P, C], F32, tag="out_sb")
    nc.vector.tensor_tensor(
        out=out_sb[:],
        in0=o_ps[:],
        in1=rb_sb[:],
        op=AOP.add)
    nc.sync.dma_start(out=out_ext[base + j * P:base + (j + 1) * P, :],
                      in_=out_sb[:])


def prep_weights(inputs):
    """Host-side folding of BN/bias into matmul weights."""
    g1, b1, m1, v1 = [np.asarray(inputs["w_bn1"][i], np.float64)
                      for i in range(4)]
    scale1 = g1 / np.sqrt(v1 + EPS)
    mean_eff = m1 - (np.asarray(inputs["bk"], np.float64)
                     - np.asarray(inputs["bq"], np.float64)
                     + np.asarray(inputs["p_b2"], np.float64)
                     + np.asarray(inputs["v_b2"], np.float64))
    shift1 = b1 - scale1 * mean_eff

    wq_s = (scale1[:, None] * inputs["Wq"]).T  # [cin, cout]
    wqn_t = np.stack([_bf(-wq_s[0:P]), _bf(-wq_s[P:2 * P])])
    wk_s = (scale1[:, None] * inputs["Wk"]).T
    wv = np.asarray(inputs["Wv"], np.float64).T
    wkv = np.concatenate([wk_s, wv], axis=1)  # [256, 512]
    wkv_t = np.stack([_bf(wkv[0:P]), _bf(wkv[P:2 * P])])

    gp, bp, mp, vp = [np.asarray(inputs["p_bn"][i], np.float64)
                      for i in range(4)]
    scale_p = gp / np.sqrt(vp + EPS)
    A1 = scale_p[:, None] * inputs["p_w1"]
    c1 = bp - scale_p * (mp - inputs["p_b1"])
    a1_t = np.zeros((64, 8, 4), np.float32)
    for u in range(8):
        a1_t[8 * u:8 * u + 3, u, :3] = A1.T

    gv, bv_, mv, vv = [np.asarray(inputs["v_bn"][i], np.float64)
                      for i in range(4)]
    scale_v = (gv / np.sqrt(vv + EPS))[0]
    sv = scale_v * inputs["v_w1"][0, 0]
    bvp = scale_v * (inputs["v_b1"][0] - mv[0]) + bv_[0]

    # w2w5[8u+k, u, cg]: rows 0-2 p_w2.T*scale1, row 3 v_w2*scale1,
    # row 4 shift1; banded per within-group index u, zero elsewhere
    w2w5 = np.zeros((32, 4, 2, P), np.float64)
    pw2s = np.asarray(inputs["p_w2"], np.float64).T * scale1[None, :]
    vw2s = np.asarray(inputs["v_w2"], np.float64)[:, 0] * scale1
    for cg in range(2):
        sl = slice(cg * P, (cg + 1) * P)
        for u in range(4):
            w2w5[8 * u + 0:8 * u + 3, u, cg] = pw2s[:, sl]
            w2w5[8 * u + 3, u, cg] = vw2s[sl]
            w2w5[8 * u + 4, u, cg] = shift1[sl]

    # w2v_r: d-major packed rows for P4: [g*CS*4] with layout (g, cs, d)
    w2v = np.zeros((4, C), np.float64)
    w2v[0:3] = np.asarray(inputs["p_w2"], np.float64).T
    w2v[3] = np.asarray(inputs["v_w2"], np.float64)[:, 0]
    w2v_r = np.ascontiguousarray(
        w2v.T.reshape(S, CS, 4).reshape(1, -1))   # [(g cs d)]

    g2, b2, m2, v2 = [np.asarray(inputs["w_bn2"][i], np.float64)
                      for i in range(4)]
    scale2 = g2 / np.sqrt(v2 + EPS)
    shift2 = b2 - scale2 * (m2 - inputs["w_b1"])
    w1s = (scale2[:, None] * inputs["w_w1"]).T  # [256, 32]
    w1_t = np.stack([_bf(w1s[0:P]), _bf(w1s[P:2 * P])])
    w2_t = _bf(np.asarray(inputs["w_w2"]).T)

    gr, br, mr, vr = [np.asarray(inputs["r_bn"][i], np.float64)
                      for i in range(4)]
    scale_r = gr / np.sqrt(vr + EPS)
    mean_r = mr - (np.asarray(inputs["bv"], np.float64)
                   + inputs["p_b2"] + inputs["v_b2"])
    shift_r = br - scale_r * mean_r
    rw2s = np.asarray(inputs["r_w"]).T
    rw2 = np.stack([_bf(rw2s[0:P]), _bf(rw2s[P:2 * P])])

    return {
        "wqn_t": wqn_t,
        "wkv_t": wkv_t,
        "w2w5": _bf(w2w5),
        "w2v_r": _bf(w2v_r),
        "w1_t": w1_t,
        "w2_t": w2_t,
        "rw2": rw2,
        "a1_t": _f32(a1_t),
        "c1_c": _f32(np.pad(np.asarray(c1, np.float64), (0, 1))[None, :]),
        "svbv_c": _f32(np.array([[sv, bvp]])),
        "sh2_c": _f32(np.asarray(shift2)[:, None]),
        "sclr_c": _f32(np.asarray(scale_r).reshape(2, P).T),
        "shfr_c": _f32(np.asarray(shift_r).reshape(2, P).T),
        "rb_c": _f32(np.asarray(inputs["r_b"])[None, :]),
    }


def prep_inputs(inputs, n_total, use_collective=False):
    """Build the per-core input maps."""
    npc = n_total // NCORES
    wd = prep_weights(inputs)
    feats_bf = _bf(inputs["feats"])                    # [N, C]
    featsT = np.ascontiguousarray(
        feats_bf.T.reshape(2, P, n_total))             # [2, 128, N]
    xyz4 = np.zeros((n_total, 4), np.float32)
    xyz4[:, :3] = inputs["xyz"]
    velw = np.ascontiguousarray(
        _f32(inputs["velocities"]).reshape(P, n_total // P))

    in_maps = []
    for c in range(NCORES):
        sl = slice(c * npc, (c + 1) * npc)
        if use_collective:
            ftb = np.ascontiguousarray(featsT[:, :, sl])
            xyzb = np.ascontiguousarray(xyz4[sl])
            velb = np.ascontiguousarray(
                _f32(inputs["velocities"][sl]).reshape(P, npc // P))
        else:
            ftb, xyzb, velb = featsT, xyz4, velw
        m = {
            "featsT_b": ftb,
            "xyz4_b": xyzb,
            "vel_b": velb,
            "featsT_own": np.ascontiguousarray(featsT[:, :, sl]),
            "featsb_sh": np.ascontiguousarray(feats_bf[sl]),
            "xyz4_sh": np.ascontiguousarray(xyz4[sl]),
            "idx_sh": np.ascontiguousarray(inputs["idx"][sl], np.int32),
            "idxv_sh": np.ascontiguousarray(inputs["idx_v"][sl], np.int32),
            "idxvf_sh": wrap_fat_idx(inputs["idx_v"][sl]),
            "voh_sh": onehot64(inputs["idx_v"][sl]),
        }
        m.update(wd)
        in_maps.append(m)
    return in_maps


def wrap_fat_idx(idxv_shard):
    """Per-tile wrapped int16 layout of idx_v//64 for dma_gather."""
    npc_l = idxv_shard.shape[0]
    nt_l = npc_l // P
    ne = NS * P
    fat = (np.asarray(idxv_shard, np.int64) // 64).astype(np.int16)
    out = np.empty((nt_l, P, ne // 16), np.int16)
    for t in range(nt_l):
        flat = fat[t * P:(t + 1) * P].T.ravel()    # e = s*128 + p
        wrap = flat.reshape(ne // 16, 16).T         # [16, ne//16]
        out[t] = np.tile(wrap, (8, 1))
    return np.ascontiguousarray(out)


def onehot64(idxv_shard):
    """bf16 one-hot of idx_v % 64, [npc, NS*64]."""
    m = (np.asarray(idxv_shard, np.int64) % 64)
    npc_l = m.shape[0]
    oh = np.zeros((npc_l, NS, 64), np.float32)
    np.put_along_axis(oh, m[:, :, None], 1.0, axis=2)
    return _bf(oh.reshape(npc_l, NS * 64))


_PROGRAM_CACHE = {}


def get_program(n_total, use_collective=False):
    key = (n_total, use_collective)
    if key not in _PROGRAM_CACHE:
        _PROGRAM_CACHE[key] = build_program(n_total, use_collective)
    return _PROGRAM_CACHE[key]


def _pjrt_pieces(nc):
    import concourse.mybir as mb

    partition_name = (nc.partition_id_tensor.name
                      if nc.partition_id_tensor else None)
    in_names, out_names, out_avals, zero_outs = [], [], [], []
    import jax

    for alloc in nc.m.functions[0].allocations:
        if not isinstance(alloc, mb.MemoryLocationSet):
            continue
        name = alloc.memorylocations[0].name
        if alloc.kind == "ExternalInput":
            if name != partition_name:
                in_names.append(name)
        elif alloc.kind == "ExternalOutput":
            shape = tuple(alloc.tensor_shape)
            dtype = mb.dt.np(alloc.dtype)
            out_names.append(name)
            out_avals.append(jax.core.ShapedArray(shape, dtype))
            zero_outs.append(np.zeros(shape, dtype))
    return partition_name, in_names, out_names, out_avals, zero_outs


def run_timed(inputs, n_total, n_iter=8, use_collective=False):
    """Run on the 8 cores; returns (out, per-exec ns from chained timing)."""
    import time as _time

    import jax
    from jax.sharding import Mesh, PartitionSpec
    from jax.experimental.shard_map import shard_map

    from concourse import bass2jax
    from concourse.bass2jax import _bass_exec_p, partition_id_tensor

    inputs = {k: np.asarray(v) for k, v in inputs.items()}
    nc = get_program(n_total, use_collective)
    in_maps = prep_inputs(inputs, n_total, use_collective)
    bass2jax.install_neuronx_cc_hook()

    (partition_name, in_names, out_names, out_avals,
     zero_outs) = _pjrt_pieces(nc)
    n_params = len(in_names)
    n_outs = len(out_names)
    all_in_names = in_names + out_names
    if partition_name is not None:
        all_in_names = all_in_names + [partition_name]

    def body_n(n_chain):
        def _body(*args):
            ins = list(args[:n_params])
            zouts = list(args[n_params:n_params + n_outs])
            outs = None
            for _ in range(n_chain):
                ops = ins + zouts
                if partition_name is not None:
                    ops = ops + [partition_id_tensor()]
                outs = list(_bass_exec_p.bind(
                    *ops,
                    out_avals=tuple(out_avals),
                    in_names=tuple(all_in_names),
                    out_names=tuple(out_names),
                    lowering_input_output_aliases=(),
                    sim_require_finite=True,
                    sim_require_nnan=True,
                    nc=nc,
                ))
            return tuple(outs)
        return _body

    devices = jax.devices()[:NCORES]
    mesh = Mesh(np.asarray(devices), ("core",))
    in_specs = (PartitionSpec("core"),) * (n_params + n_outs)
    out_specs = (PartitionSpec("core"),) * n_outs
    per_core = [[np.asarray(m[name]) for name in in_names]
                for m in in_maps]
    concat_in = [np.concatenate([per_core[c][i] for c in range(NCORES)],
                                axis=0) for i in range(n_params)]
    concat_zero = [np.concatenate([z] * NCORES, axis=0) for z in zero_outs]
    args = [jax.device_put(a) for a in concat_in + concat_zero]

    # correctness run (single execution)
    fn1 = jax.jit(shard_map(body_n(1), mesh=mesh, in_specs=in_specs,
                            out_specs=out_specs, check_rep=False),
                  keep_unused=True)
    res = jax.block_until_ready(fn1(*args))
    out_full = np.asarray(res[out_names.index("out")])

    exec_ns = None
    if n_iter > 1:
        fnN = jax.jit(shard_map(body_n(n_iter), mesh=mesh,
                                in_specs=in_specs, out_specs=out_specs,
                                check_rep=False), keep_unused=True)
        r = jax.block_until_ready(fnN(*args))   # warmup/compile
        tN = None
        for _ in range(3):
            t1 = _time.perf_counter()
            r = jax.block_until_ready(fnN(*args))
            dt = _time.perf_counter() - t1
            tN = dt if tN is None else min(tN, dt)
        t1w = None
        for _ in range(3):
            t1 = _time.perf_counter()
            jax.block_until_ready(fn1(*args))
            dt = _time.perf_counter() - t1
            t1w = dt if t1w is None else min(t1w, dt)
        per = (tN - t1w) / (n_iter - 1)
        exec_ns = int(max(per, tN / n_iter) * 1e9) if per <= 0 else int(per * 1e9)
    return out_full, exec_ns


def kernel(**inputs):
    inputs = {k: np.asarray(v) for k, v in inputs.items()}
    n_total = inputs["feats"].shape[0]
    out, _ = run_timed(inputs, n_total, n_iter=1)
    return np.ascontiguousarray(out, dtype=np.float32)



# revision 13
# speedup vs baseline: 1.3078x; 1.3078x over previous
"""DopplerPTNet point-transformer block on 8 Trainium2 NeuronCores — v2.

Strategy (point-parallel, replicated k/v table build):
  - Each core owns N/8 query points.  Every core builds the FULL [N, 388]
    bf16 "kv table" locally from replicated transposed features (no
    collective): one row per point = [ k(256 fp8e4m3, pre-scaled by bn1
    gamma/rsqrt(var)) | v(256 bf16) | tg(3 bf16) ], tg = A1@xyz + c1 being
    the position encoder's first affine.  A separate [N] bf16 vh table
    holds the velocity encoding scalar (post BN+ReLU).
  - Main pass per 128-point tile (4-tile chunks): ONE merged indirect DMA
    gathers 64 x 16 kv rows point-major into SBUF; a second indirect DMA
    gathers vh scalars straight into the geometry tile.  The attention
    logit MLP runs channel-major: k reaches channel-major via PE-transpose
    matmuls accumulated directly into the logit PSUM (no SBUF transpose
    pass), the position/velocity encoding enters via K=5 matmuls against a
    transposed geometry tile whose 5th row adds the folded BN shift, and q
    is subtracted via an identity matmul of the pre-negated q.  Softmax and
    the share-grouped aggregation run point-major on DVE/Pool; the position
    encoding part of the aggregated values is reconstructed from the
    attention-weighted geometry moments (U-trick) instead of materializing
    [N,NS,C] values.

All BatchNorms are inference-affine and folded on the host.  w_b2 is
dropped (softmax-invariant); bk-bq+p_b2+v_b2 folds into shift1; bv+p_b2+
v_b2 folds into r_bn's mean because sum(attn)==1.
"""

import sys

sys.path.insert(0, "/opt/trn_rl_repo")

import numpy as np
import ml_dtypes

import concourse.bass as bass
import concourse.mybir as mybir
import concourse.tile as tile
from concourse import bacc
from concourse.bass import IndirectOffsetOnAxis
from concourse.masks import make_identity

BF16 = mybir.dt.bfloat16
F32 = mybir.dt.float32
F8 = mybir.dt.float8e4
I32 = mybir.dt.int32
AOP = mybir.AluOpType
AFT = mybir.ActivationFunctionType

NCORES = 8
C = 256
NS = 16
CS = 32
S = 8
P = 128
ROW = 388          # bf16 elems per kv row: k 128(=256 fp8) | v 256 | tg 3 | pad
EPS = 1e-5
CH = 2             # tiles per phase-D chunk
BCH = 2            # tiles per phase-B chunk


def _bf(x):
    return np.ascontiguousarray(
        np.asarray(x, dtype=np.float32).astype(ml_dtypes.bfloat16))


def _f32(x):
    return np.ascontiguousarray(x, dtype=np.float32)


def build_program(n_total: int, use_collective: bool = False, phases: str = "all", taps: bool = False,
                  skip: frozenset = frozenset(), table_io: str | None = None):
    npc = n_total // NCORES
    nt = npc // P                 # phase-D tiles per core
    nb = npc if use_collective else n_total
    ntb = nb // P                 # phase-B tiles per core

    nc = bacc.Bacc(
        "TRN2",
        target_bir_lowering=False,
        debug=False,
        enable_asserts=False,
        num_devices=NCORES,
    )

    # ---- I/O -----------------------------------------------------------
    def inp(name, shape, dt):
        return nc.dram_tensor(name, shape, dt, kind="ExternalInput")

    # phase-B inputs (full N if replicated, own shard if collective)
    featsT_b = inp("featsT_b", [2, P, nb], BF16)
    xyz4_b = inp("xyz4_b", [nb, 4], F32)
    vel_b = inp("vel_b", [P, nb // P], F32)
    # phase-D per-core inputs
    featsT_own = inp("featsT_own", [2, P, npc], BF16)
    featsb_sh = inp("featsb_sh", [npc, C], BF16)
    xyz4_sh = inp("xyz4_sh", [npc, 4], F32)
    idx_sh = inp("idx_sh", [npc, NS], I32)
    idxv_sh = inp("idxv_sh", [npc, NS], I32)
    idxvf_sh = inp("idxvf_sh", [npc // P, P, NS * P // 16], mybir.dt.int16)
    voh_sh = inp("voh_sh", [npc, NS * 64], BF16)   # one-hot of idx_v % 64
    # folded weights
    wqn_t = inp("wqn_t", [2, P, C], BF16)      # -(scale1*Wq).T cin-groups
    wkv_t = inp("wkv_t", [2, P, 2 * C], BF16)  # [(scale1*Wk).T | Wv.T]
    w2w5 = inp("w2w5", [32, 4, 2, P], BF16)    # 8-banded [pw2 s1;vw2 s1;sh1]
    w2v_r = inp("w2v_r", [1, 4 * C], BF16)     # pe rhs rows (unscaled)
    w1_t = inp("w1_t", [2, P, CS], BF16)       # (scale2*w_w1).T
    w2_t = inp("w2_t", [CS, CS], BF16)         # w_w2.T
    rw2 = inp("rw2", [2, P, C], BF16)          # r_w.T
    a1_t = inp("a1_t", [64, 8, 4], F32)        # A1.T 8-banded
    c1_c = inp("c1_c", [1, 4], F32)            # pos affine bias
    svbv_c = inp("svbv_c", [1, 2], F32)        # velocity scale/bias
    sh2_c = inp("sh2_c", [CS, 1], F32)         # bn2 shift per cs
    sclr_c = inp("sclr_c", [P, 2], F32)        # rho bn scale (ch-major)
    shfr_c = inp("shfr_c", [P, 2], F32)        # rho bn shift (ch-major)
    rb_c = inp("rb_c", [1, C], F32)            # r_b

    out_ext = nc.dram_tensor("out", [npc, C], F32, kind="ExternalOutput")
    tap_t = {}
    if taps:
        for nm, shp, dt in [
            ("tap_stag", [P, NS * ROW], BF16),
            ("tap_pmx", [P, NS * 8], BF16),
            ("tap_T2", [32, 4 * P], BF16),
            ("tap_qn", [P, 2 * P], BF16),
            ("tap_t1", [P, 2 * NS * P], BF16),
            ("tap_t2o", [CS, NS * P], BF16),
            ("tap_attnE", [P, NS * CS], BF16),
            ("tap_rin", [P, C], BF16),
            ("tap_kv", [P, 4 * ROW], BF16),
            ("tap_vh", [P, 16], BF16),
        ]:
            tap_t[nm] = nc.dram_tensor(nm, shp, dt, kind="ExternalOutput")

    # ---- internal DRAM -------------------------------------------------
    if use_collective:
        kv_shard = nc.dram_tensor("kv_shard", [npc, ROW], BF16)
        vh_shard = nc.dram_tensor("vh_shard", [npc, 1], F32)
        kv = nc.dram_tensor("kv_full", [n_total, ROW], BF16,
                            addr_space="Shared")
        vh = nc.dram_tensor("vh_full", [n_total, 1], F32,
                            addr_space="Shared")
    else:
        kv_shard, vh_shard = None, None
        tkind = {"out": "ExternalOutput", "in": "ExternalInput"}.get(
            table_io)
        if tkind is not None:
            kv = nc.dram_tensor("kv_full", [n_total, ROW], BF16, kind=tkind)
            vh = nc.dram_tensor("vh_full", [n_total, 1], F32, kind=tkind)
        else:
            kv = nc.dram_tensor("kv_full", [n_total, ROW], BF16)
            vh = nc.dram_tensor("vh_full", [n_total, 1], F32)
    kv_read, vh_read = kv, vh
    if "dummykv" in skip:
        kv_read = nc.dram_tensor("kv_dummy", [n_total, ROW], BF16,
                                 kind="ExternalInput")
    if "dummyvh" in skip:
        vh_read = nc.dram_tensor("vh_dummy", [n_total, 1], F32,
                                 kind="ExternalInput")

    rg = [list(range(NCORES))]

    with tile.TileContext(nc) as tc:
        with (
            tc.tile_pool(name="const", bufs=1) as cpool,
            tc.tile_pool(name="work", bufs=2) as pool,
            tc.tile_pool(name="big", bufs=2) as bigpool,
            tc.tile_pool(name="ps_t1", bufs=2, space="PSUM") as ps_t1,
            tc.tile_pool(name="ps_t2", bufs=2, space="PSUM") as ps_t2,
            tc.tile_pool(name="ps_e", bufs=2, space="PSUM") as ps_e,
            tc.tile_pool(name="ps_l", bufs=2, space="PSUM") as ps_l,
        ):
            # ---------- constants ----------
            ident_b = cpool.tile([P, P], BF16, tag="ident_b")
            make_identity(nc, ident_b[:])
            ident_f = cpool.tile([P, P], F32, tag="ident_f")
            make_identity(nc, ident_f[:])

            def cload(src, shape, dt, tag):
                t = cpool.tile(shape, dt, tag=tag)
                nc.sync.dma_start(out=t[:], in_=src)
                return t

            def gload(src, width, tag):
                t = cpool.tile([P, 2, width], BF16, tag=tag)
                for g in range(2):
                    nc.sync.dma_start(out=t[:, g, :], in_=src[g, :, :])
                return t

            wqn_sb = gload(wqn_t, C, "wqn")
            wkv_sb = gload(wkv_t, 2 * C, "wkv")
            w1_sb = gload(w1_t, CS, "w1")
            rw2_sb = gload(rw2, C, "rw2")
            w2w5_sb = cload(w2w5[:, :, :, :], [32, 4, 2, P], BF16, "w2w5")
            w2_sb = cload(w2_t[:, :], [CS, CS], BF16, "w2")
            a1_sb = cload(a1_t[:, :, :], [64, 8, 4], F32, "a1")
            sh2_sb = cload(sh2_c[:, :], [CS, 1], F32, "sh2")
            sclr_sb = cload(sclr_c[:, :], [P, 2], F32, "sclr")
            shfr_sb = cload(shfr_c[:, :], [P, 2], F32, "shfr")

            def bcast(name, src, width, dt):
                row = cpool.tile([1, width], dt, tag=name + "r")
                nc.sync.dma_start(out=row[:], in_=src)
                full = cpool.tile([P, width], dt, tag=name)
                nc.gpsimd.partition_broadcast(full[:], row[:])
                return full

            svbv_sb = bcast("svbv", svbv_c[:, :], 2, F32)
            c1_sb = bcast("c1", c1_c[:, :], 4, F32)
            w2vR = bcast("w2vR", w2v_r[:, :], 4 * C, BF16)
            rb_sb = bcast("rb", rb_c[:, :], C, F32)

            # ---------- phase V: velocity table (bulk) ----------
            nvb = (nb // P) if phases in ("all", "VB") else 0
            for lo in range(0, nvb, 256):
                w = min(256, nvb - lo)
                vel_t = bigpool.tile([P, w], F32, tag="vel_t")
                nc.sync.dma_start(out=vel_t[:], in_=vel_b[:, lo:lo + w])
                vhb = bigpool.tile([P, w], F32, tag="vhb")
                nc.scalar.activation(
                    vhb[:], vel_t[:], AFT.Relu,
                    bias=svbv_sb[:, 1:2], scale=svbv_sb[:, 0:1],
                )
                dst = vh_shard if use_collective else vh
                nc.sync.dma_start(
                    out=dst[:, :]
                    .rearrange("(p j) o -> p (j o)", p=P)[:, lo:lo + w],
                    in_=vhb[:],
                )

            # ---------- phase B: kv table build (skewed pipeline) ----------
            kv_dst = kv_shard if use_collective else kv
            XB = 8                    # xyz tiles per transpose batch
            if phases == "D":
                ntb = 0

            def b_loads(cc):
                ld = {"cc": cc}
                if cc % XB == 0:
                    xw = min(XB, ntb - cc)
                    xyz_t = pool.tile([P, XB, 8], F32, tag="xyz_b")
                    nc.vector.memset(xyz_t[:, :, 4:8], 0)
                    if xw < XB:
                        nc.vector.memset(xyz_t[:, xw:, 0:4], 0)
                    nc.sync.dma_start(
                        out=xyz_t[:, :xw, 0:4],
                        in_=xyz4_b[cc * P:(cc + xw) * P, :]
                        .rearrange("(j p) d -> p j d", p=P),
                    )
                    ld["xyz_t"] = xyz_t
                    ld["xw"] = xw
                ftT = pool.tile([P, 2, BCH * P], BF16, tag="ftT_b")
                nc.sync.dma_start(
                    out=ftT[:],
                    in_=featsT_b[:, :, cc * P:(cc + BCH) * P]
                    .rearrange("g p n -> p g n"),
                )
                ld["ftT"] = ftT
                return ld

            batch_state = {}

            def b_compute(ld):
                cc = ld["cc"]
                if "xyz_t" in ld:
                    xw = ld["xw"]
                    xT_ps = ps_e.tile([8 * XB, P], F32, tag="ps_e")
                    nc.tensor.transpose(
                        out=xT_ps[:],
                        in_=ld["xyz_t"][:, :, :]
                        .rearrange("p j d -> p (j d)"),
                        identity=ident_f[:],
                    )
                    xT_sb = pool.tile([8 * XB, P], F32, tag="xT_b")
                    nc.scalar.copy(xT_sb[:], xT_ps[:])
                    tg_ps = ps_e.tile([P, XB, 4], F32, tag="ps_e")
                    for j in range(xw):
                        nc.tensor.matmul(
                            out=tg_ps[:, j, :],
                            lhsT=xT_sb[:, :],
                            rhs=a1_sb[:, j, :],
                            start=True, stop=True,
                        )
                    tg_sb = pool.tile([P, XB, 4], BF16, tag="tg_b")
                    nc.vector.tensor_tensor(
        out=tg_sb[:, :xw, :],
        in0=tg_ps[:, :xw, :],
        in1=c1_sb[:, :4]
                        .rearrange("p (o d) -> p o d", o=1)
                        .to_broadcast([P, xw, 4]),
        op=AOP.add)
                    batch_state["tg_sb"] = tg_sb
                    batch_state["c0"] = cc
                ftT = ld["ftT"]
                base = cc * P
                row_t = pool.tile([P, BCH, ROW], BF16, tag="row_t")
                row_f8 = row_t[:, :, 0:P].bitcast(F8)
                for j in range(BCH):
                    kv_ps = ps_t1.tile([P, 2 * C], F32, tag="ps_t1")
                    for g in range(2):
                        nc.tensor.matmul(
                            out=kv_ps[:],
                            lhsT=ftT[:, g, j * P:(j + 1) * P],
                            rhs=wkv_sb[:, g, :],
                            start=(g == 0), stop=(g == 1),
                        )
                    nc.scalar.copy(row_f8[:, j, :], kv_ps[:, 0:C])
                    nc.vector.tensor_copy(
                        row_t[:, j, P:P + C], kv_ps[:, C:2 * C])
                co = cc - batch_state["c0"]
                nc.vector.tensor_copy(
                    row_t[:, :, P + C:P + C + 4],
                    batch_state["tg_sb"][:, co:co + BCH, :],
                )
                nc.sync.dma_start(
                    out=kv_dst[base:base + BCH * P, :]
                    .rearrange("(j p) r -> p j r", p=P),
                    in_=row_t[:],
                )

            prev = None
            for cc in range(0, ntb, BCH):
                cur = b_loads(cc)
                if prev is not None:
                    b_compute(prev)
                prev = cur
            if prev is not None:
                b_compute(prev)

            # ---------- phase C: all-gather tables ----------
            if use_collective:
                nc.gpsimd.collective_compute(
                    "AllGather", AOP.bypass, replica_groups=rg,
                    ins=[kv_shard.ap().opt()], outs=[kv.ap().opt()],
                )
                nc.gpsimd.collective_compute(
                    "AllGather", AOP.bypass, replica_groups=rg,
                    ins=[vh_shard.ap().opt()], outs=[vh.ap().opt()],
                )

            # ---------- phase D: main pass (skewed CH-tile chunks) ------
            if taps:
                kvt = pool.tile([P, 4, ROW], BF16, tag="kvt")
                nc.sync.dma_start(
                    out=kvt[:],
                    in_=kv[0:4 * P, :].rearrange("(j p) r -> p j r", p=P))
                nc.sync.dma_start(
                    out=tap_t["tap_kv"][:, :],
                    in_=kvt[:].rearrange("p j r -> p (j r)"))
                vht = pool.tile([P, 16], BF16, tag="vht")
                nc.sync.dma_start(
                    out=vht[:],
                    in_=vh[0:2048, :].rearrange("(p j) o -> p (j o)", p=P))
                nc.sync.dma_start(out=tap_t["tap_vh"][:, :], in_=vht[:])
            if phases == "VB":
                nt = 0

            def d_loads(tc0):
                cw = min(CH, nt - tc0)
                rsl = slice(tc0 * P, (tc0 + cw) * P)
                ld = {"tc0": tc0, "cw": cw}
                idx_t = pool.tile([P, CH, NS], I32, tag="idx_t")
                nc.sync.dma_start(
                    out=idx_t[:, :cw, :],
                    in_=idx_sh[rsl, :].rearrange("(j p) s -> p j s", p=P),
                )
                idxv_t = pool.tile([P, CH, NS], I32, tag="idxv_t")
                nc.sync.dma_start(
                    out=idxv_t[:, :cw, :],
                    in_=idxv_sh[rsl, :].rearrange("(j p) s -> p j s", p=P),
                )
                xyz_t = pool.tile([P, CH, 8], F32, tag="xyz_t")
                nc.vector.memset(xyz_t[:, :, 4:8], 0)
                nc.sync.dma_start(
                    out=xyz_t[:, :cw, 0:4],
                    in_=xyz4_sh[rsl, :].rearrange("(j p) d -> p j d", p=P),
                )
                fpm = pool.tile([P, CH, C], BF16, tag="fpm")
                nc.sync.dma_start(
                    out=fpm[:, :cw, :],
                    in_=featsb_sh[rsl, :].rearrange("(j p) c -> p j c", p=P),
                )
                ftT4 = pool.tile([P, 2, CH * P], BF16, tag="ftT4")
                nc.sync.dma_start(
                    out=ftT4[:, :, : cw * P],
                    in_=featsT_own[:, :, rsl].rearrange("g p n -> p g n"),
                )
                stag = bigpool.tile([P, CH, NS, ROW], BF16, tag="stag")
                pmx = pool.tile([P, CH, NS, 8], BF16, tag="pmx")
                nc.vector.memset(pmx[:, :, :, 4:5], 1.0)
                nc.vector.memset(pmx[:, :, :, 5:8], 0)
                if "kvgather" not in skip:
                    for jj in range(cw):
                        for s in range(NS):
                            nc.gpsimd.indirect_dma_start(
                                out=stag[:, jj, s, :],
                                out_offset=None,
                                in_=kv_read[:, :],
                                in_offset=IndirectOffsetOnAxis(
                                    ap=idx_t[:, jj, s:s + 1], axis=0),
                            )
                idxvf_t = pool.tile([P, CH, NS * P // 16], mybir.dt.int16,
                                    tag="idxvf_t")
                nc.sync.dma_start(
                    out=idxvf_t[:, :cw, :],
                    in_=idxvf_sh[tc0:tc0 + cw, :, :]
                    .rearrange("j p w -> p j w"),
                )
                velf = bigpool.tile([P, CH, NS, 64], F32, tag="velf")
                voh = bigpool.tile([P, CH, NS * 64], BF16, tag="voh")
                nc.sync.dma_start(
                    out=voh[:, :cw, :],
                    in_=voh_sh[rsl, :].rearrange("(j p) w -> p j w", p=P),
                )
                if "velgather" not in skip:
                    nc.gpsimd.dma_gather(
                        out_ap=velf[:, :cw, :, :]
                        .rearrange("p j s k -> p (j s) k"),
                        in_ap=vh_read[:, :].rearrange("(m k) o -> m (k o)", k=64),
                        idxs_ap=idxvf_t[:, :cw, :]
                        .rearrange("p j w -> p (j w)"),
                        num_idxs=cw * NS * P,
                        num_idxs_reg=cw * NS * P,
                        elem_size=64,
                        transpose=False,
                        single_packet=False,
                    )
                else:
                    nc.vector.memset(
                        velf[:, :cw, :, :].rearrange("p j s k -> p (j s k)"),
                        0)
                ld_extra = {"velf": velf, "voh": voh}
                ld.update(xyz_t=xyz_t, fpm=fpm, ftT4=ftT4, stag=stag,
                          pmx=pmx, **ld_extra)
                return ld

            def d_compute(ld):
                tc0, cw = ld["tc0"], ld["cw"]
                xyz_t, fpm = ld["xyz_t"], ld["fpm"]
                ftT4, stag, pmx = ld["ftT4"], ld["stag"], ld["pmx"]
                base = tc0 * P
                xa_ps = ps_e.tile([P, 128], F32, tag="ps_e")
                xT_ps = xa_ps[0:CH * 8, 0:P]
                nc.tensor.transpose(
                    out=xT_ps,
                    in_=xyz_t[:, :, :].rearrange("p j d -> p (j d)"),
                    identity=ident_f[:],
                )
                xT_sb = pool.tile([CH * 8, P], F32, tag="xT_sb")
                nc.scalar.copy(xT_sb[:], xT_ps)
                axc_ps = ps_e.tile([P, CH, 4], F32, tag="ps_e")
                for j in range(cw):
                    nc.tensor.matmul(
                        out=axc_ps[:, j, :3],
                        lhsT=xT_sb[:, :],
                        rhs=a1_sb[0:CH * 8, j, :3],
                        start=True, stop=True,
                    )
                nc.vector.tensor_tensor(
                    out=pmx[:, :cw, :, 0:3],
                    in0=stag[:, :cw, :, P + C:P + C + 3],
                    in1=axc_ps[:, :cw, :3]
                    .rearrange("p j (o d) -> p j o d", o=1)
                    .to_broadcast([P, cw, NS, 3]),
                    op=AOP.subtract,
                )
                nc.vector.tensor_scalar_max(
                    pmx[:, :cw, :, 0:3], pmx[:, :cw, :, 0:3], 0.0)
                velf, voh = ld["velf"], ld["voh"]
                vprod = bigpool.tile([P, CH, NS, 64], BF16, tag="vprod")
                nc.vector.tensor_tensor(
                    out=vprod[:, :cw, :, :],
                    in0=velf[:, :cw, :, :],
                    in1=voh[:, :cw, :].rearrange("p j (s k) -> p j s k",
                                                 k=64),
                    op=AOP.mult,
                )
                with nc.allow_low_precision(reason="one-hot select sum"):
                    nc.vector.tensor_reduce(
                        out=pmx[:, :cw, :, 3:4],
                        in_=vprod[:, :cw, :, :],
                        axis=mybir.AxisListType.X,
                        op=AOP.add,
                    )
                if "compute" in skip:
                    return
                for j in range(cw):
                    _tile_body(nc, ps_t1, ps_t2, ps_e, ps_l, pool, bigpool,
                               ident_b, wqn_sb, w2w5_sb, w1_sb, w2_sb,
                               sh2_sb, w2vR, sclr_sb, shfr_sb, rb_sb,
                               rw2_sb, stag, pmx, ftT4, fpm, out_ext,
                               base, j,
                               tap_t if (tc0 == 0 and j == 0) else None)

            prev = None
            for tc0 in range(0, nt, CH):
                cur = d_loads(tc0)
                if prev is not None:
                    d_compute(prev)
                prev = cur
            if prev is not None:
                d_compute(prev)

    nc.compile()
    return nc


def _tile_body(nc, ps_t1, ps_t2, ps_e, ps_l, pool, bigpool, ident_b, wqn_sb,
               w2w5_sb, w1_sb, w2_sb, sh2_sb, w2vR, sclr_sb, shfr_sb,
               rb_sb, rw2_sb, stag, pmx, ftT4, fpm, out_ext, base, j,
               tap_t=None):
    def tap(nm, ap):
        if tap_t is not None and nm in tap_t:
            nc.sync.dma_start(out=tap_t[nm][:, :], in_=ap)
    """Compute one 128-point tile (index j within the current chunk)."""
    stag_f8 = stag[:, j, :, 0:P].bitcast(F8)        # [P, NS, 256] fp8 k
    v_view = stag[:, j, :, P:P + C]                 # [P, NS, 256] bf16 v

    # ---- q (negated, channel-major) ----
    q_ps = ps_e.tile([P, 2, P], F32, tag="ps_e")
    for cg in range(2):
        for g in range(2):
            nc.tensor.matmul(
                out=q_ps[:, cg, :],
                lhsT=wqn_sb[:, g, cg * P:(cg + 1) * P],
                rhs=ftT4[:, g, j * P:(j + 1) * P],
                start=(g == 0), stop=(g == 1),
            )
    qn_sb = pool.tile([P, 2, P], BF16, tag="qn_sb")
    nc.scalar.copy(qn_sb[:], q_ps[:])
    tap("tap_stag", stag[:, j, :, :].rearrange("p s r -> p (s r)"))
    tap("tap_pmx", pmx[:, j, :, :].rearrange("p s d -> p (s d)"))
    tap("tap_qn", qn_sb[:, :, :].rearrange("p g n -> p (g n)"))
    q_rep = pool.tile([P, 2, 4 * P], BF16, tag="q_rep")
    for cg in range(2):
        nc.vector.tensor_copy(
            q_rep[:, cg, :],
            qn_sb[:, cg, :].rearrange("p (o n) -> p o n", o=1)
            .to_broadcast([P, 4, P]),
        )

    # ---- geometry transpose: pmx[j] -> T2 [4 groups][8u+d, pt] ----
    t2g_ps = ps_e.tile([32, 4, P], BF16, tag="ps_e")
    for g in range(4):
        nc.tensor.transpose(
            out=t2g_ps[:, g, :],
            in_=pmx[:, j, 4 * g:4 * g + 4, :].rearrange("p s d -> p (s d)"),
            identity=ident_b[:],
        )
    T2 = pool.tile([32, 4, P], BF16, tag="T2")
    nc.scalar.copy(T2[:], t2g_ps[:])
    tap("tap_T2", T2[:, :, :].rearrange("k g n -> k (g n)"))

    # ---- t1 = relu(k' - q' + pe' + shift1), channel-major ----
    t1 = bigpool.tile([P, 2, NS * P], BF16, tag="t1")
    for cg in range(2):
        for h in range(4):
            wps = ps_t1.tile([P, 4 * P], F32, tag="ps_t1")
            nc.tensor.matmul(
                out=wps[:],
                lhsT=ident_b[:],
                rhs=q_rep[:, cg, :],
                start=True, stop=False,
                skip_group_check=True,
            )
            for s4 in range(4):
                s = h * 4 + s4
                nc.tensor.matmul(
                    out=wps[:, s4 * P:(s4 + 1) * P],
                    lhsT=w2w5_sb[:, s % 4, cg, :],
                    rhs=T2[:, s // 4, :],
                    start=False, stop=False,
                    skip_group_check=True,
                )
                nc.tensor.matmul(
                    out=wps[:, s4 * P:(s4 + 1) * P],
                    lhsT=stag_f8[:, s, cg * P:(cg + 1) * P],
                    rhs=ident_b[:],
                    start=False, stop=(s4 == 3),
                    skip_group_check=True,
                )
            nc.scalar.activation(
                t1[:, cg, h * 4 * P:(h + 1) * 4 * P], wps[:], AFT.Relu)

    tap("tap_t1", t1[:, :, :].rearrange("p g n -> p (g n)"))
    # ---- W1 (256->32) + bn2 + relu ----
    t2 = pool.tile([CS, NS * P], BF16, tag="t2")
    for q4 in range(4):
        w1_ps = ps_t2.tile([CS, 4 * P], F32, tag="ps_t2")
        for cg in range(2):
            nc.tensor.matmul(
                out=w1_ps[:],
                lhsT=w1_sb[:, cg, :],
                rhs=t1[:, cg, q4 * 4 * P:(q4 + 1) * 4 * P],
                start=(cg == 0), stop=(cg == 1),
            )
        nc.scalar.activation(
            t2[:, q4 * 4 * P:(q4 + 1) * 4 * P], w1_ps[:],
            AFT.Relu, bias=sh2_sb[:, 0:1], scale=1.0,
        )

    tap("tap_t2o", t2[:, :])
    # ---- W2 -> point-major logits, exp ----
    attn_ps = ps_t2.tile([P, NS * CS], F32, tag="ps_t2")  # shares t2 slots
    for s in range(NS):
        nc.tensor.matmul(
            out=attn_ps[:, s * CS:(s + 1) * CS],
            lhsT=t2[:, s * P:(s + 1) * P],
            rhs=w2_sb[:, :],
            start=True, stop=True,
        )
    attnE = pool.tile([P, NS * CS], BF16, tag="attnE")
    nc.scalar.activation(attnE[:], attn_ps[:], AFT.Exp)

    tap("tap_attnE", attnE[:, :])
    # ---- softmax over ns ----
    scr = pool.tile([P, 12 * CS], BF16, tag="scr")
    v0 = attnE[:, :].rearrange("p (s c) -> p s c", c=CS)
    r1 = scr[:, 0:8 * CS].rearrange("p (s c) -> p s c", c=CS)
    nc.vector.tensor_tensor(
        out=r1, in0=v0[:, 0:16:2, :], in1=v0[:, 1:16:2, :],
        op=AOP.add)
    r2 = scr[:, 8 * CS:12 * CS].rearrange("p (s c) -> p s c", c=CS)
    nc.vector.tensor_tensor(
        out=r2,
        in0=r1[:, 0:8:2, :],
        in1=r1[:, 1:8:2, :],
        op=AOP.add)
    ssum = pool.tile([P, 3 * CS], F32, tag="ssum")
    s3 = ssum[:, CS:3 * CS].rearrange("p (s c) -> p s c", c=CS)
    nc.vector.tensor_tensor(
        out=s3,
        in0=r2[:, 0:4:2, :],
        in1=r2[:, 1:4:2, :],
        op=AOP.add)
    nc.vector.tensor_tensor(
        out=ssum[:, 0:CS].rearrange("p (o c) -> p o c", o=1),
        in0=s3[:, 0:1, :],
        in1=s3[:, 1:2, :],
        op=AOP.add)
    rcp = pool.tile([P, 2 * CS], F32, tag="rcp")
    nc.vector.reciprocal(rcp[:, 0:CS], ssum[:, 0:CS])
    rcpb = pool.tile([P, CS], BF16, tag="rcpb")
    nc.vector.tensor_copy(rcpb[:], rcp[:, 0:CS])
    attn_n = pool.tile([P, NS * CS], BF16, tag="attn_n")
    nc.vector.tensor_tensor(
        out=attn_n[:].rearrange("p (s c) -> p s c", c=CS),
        in0=attnE[:].rearrange("p (s c) -> p s c", c=CS),
        in1=rcpb[:].rearrange("p (o c) -> p o c", o=1)
        .to_broadcast([P, NS, CS]),
        op=AOP.mult)

    # ---- U-trick: attention-weighted geometry moments ----
    # Umul[p, s, cs, d] = attn_n[p,s,cs] * pmx[p,j,s,d]   (d = 0..3)
    um = bigpool.tile([P, NS, CS, 4], BF16, tag="um")
    for d in range(4):
        eng = nc.vector
        eng.tensor_tensor(
            out=um[:, :, :, d:d + 1],
            in0=attn_n[:].rearrange("p (s c o) -> p s c o", c=CS, o=1),
            in1=pmx[:, j, :, d:d + 1]
            .rearrange("p s (o d) -> p s o d", o=1)
            .to_broadcast([P, NS, CS, 1]),
            op=AOP.mult,
        )
    # tree-reduce over ns (views of packed 128-wide rows)
    umv = um[:, :, :, :].rearrange("p s c d -> p s (c d)")
    W4 = CS * 4
    ut = pool.tile([P, 15 * W4], BF16, tag="ut")
    u1 = ut[:, 0:8 * W4].rearrange("p (s w) -> p s w", w=W4)
    nc.vector.tensor_tensor(
        out=u1,
        in0=umv[:, 0:16:2, :],
        in1=umv[:, 1:16:2, :],
        op=AOP.add)
    u2 = ut[:, 8 * W4:12 * W4].rearrange("p (s w) -> p s w", w=W4)
    nc.vector.tensor_tensor(
        out=u2,
        in0=u1[:, 0:8:2, :],
        in1=u1[:, 1:8:2, :],
        op=AOP.add)
    u3 = ut[:, 12 * W4:14 * W4].rearrange("p (s w) -> p s w", w=W4)
    nc.vector.tensor_tensor(
        out=u3,
        in0=u2[:, 0:4:2, :],
        in1=u2[:, 1:4:2, :],
        op=AOP.add)
    u3f = ut[:, 14 * W4:15 * W4].rearrange("p (o w) -> p o w", o=1)
    nc.vector.tensor_tensor(
        out=u3f,
        in0=u3[:, 0:2:2, :],
        in1=u3[:, 1:2:2, :],
        op=AOP.add)
    # P4[p, g, cs, d] = U3[p, cs, d] * w2vR[p, (d-major rows)]
    p4 = pool.tile([P, S, CS, 4], BF16, tag="p4")
    nc.vector.tensor_tensor(
        out=p4[:],
        in0=ut[:, 14 * W4:15 * W4]
        .rearrange("p (o c d) -> p o c d", o=1, d=4)
        .to_broadcast([P, S, CS, 4]),
        in1=w2vR[:, :].rearrange("p (g c d) -> p g c d", g=S, d=4),
        op=AOP.mult)
    agg_pe = pool.tile([P, C], F32, tag="agg_pe")
    nc.vector.tensor_reduce(
        out=agg_pe[:].rearrange("p (g c o) -> p g c o", g=S, o=1),
        in_=p4[:],
        axis=mybir.AxisListType.X,
        op=AOP.add,
    )

    # ---- v aggregation: prod over share groups + ns tree ----
    prod = bigpool.tile([P, NS * C], BF16, tag="prod")
    nc.vector.tensor_tensor(
        out=prod[:].rearrange("p (s g c) -> p s g c", g=S, c=CS),
        in0=v_view.rearrange("p s (g c) -> p s g c", c=CS),
        in1=attn_n[:].rearrange("p (s o c) -> p s o c", o=1, c=CS)
        .to_broadcast([P, NS, S, CS]),
        op=AOP.mult)
    tscr = bigpool.tile([P, 14 * C], BF16, tag="tscr")
    pv = prod[:, :].rearrange("p (s c) -> p s c", c=C)
    u1v = tscr[:, 0:8 * C].rearrange("p (s c) -> p s c", c=C)
    nc.vector.tensor_tensor(
        out=u1v,
        in0=pv[:, 0:16:2, :],
        in1=pv[:, 1:16:2, :],
        op=AOP.add)
    u2v = tscr[:, 8 * C:12 * C].rearrange("p (s c) -> p s c", c=C)
    nc.vector.tensor_tensor(
        out=u2v,
        in0=u1v[:, 0:8:2, :],
        in1=u1v[:, 1:8:2, :],
        op=AOP.add)
    u3v = tscr[:, 12 * C:14 * C].rearrange("p (s c) -> p s c", c=C)
    nc.vector.tensor_tensor(
        out=u3v,
        in0=u2v[:, 0:4:2, :],
        in1=u2v[:, 1:4:2, :],
        op=AOP.add)
    aggv = pool.tile([P, C], F32, tag="aggv")
    nc.vector.tensor_tensor(
        out=aggv[:].rearrange("p (o c) -> p o c", o=1),
        in0=u3v[:, 0:1, :],
        in1=u3v[:, 1:2, :],
        op=AOP.add)

    # ---- residual (BN+relu folded into the rho activation) ----
    rin = pool.tile([P, C], BF16, tag="rin")
    nc.vector.tensor_tensor(
        out=rin[:],
        in0=aggv[:],
        in1=agg_pe[:],
        op=AOP.add)
    nc.vector.tensor_tensor(
        out=rin[:],
        in0=rin[:],
        in1=fpm[:, j, :],
        op=AOP.add)
    tap("tap_rin", rin[:])
    rT_ps = ps_l.tile([P, 2, P], BF16, tag="ps_l")
    for cg in range(2):
        nc.tensor.transpose(
            out=rT_ps[:, cg, :],
            in_=rin[:, cg * P:(cg + 1) * P],
            identity=ident_b[:],
        )
    rT_sb = pool.tile([P, 2, P], BF16, tag="rT_sb")
    for cg in range(2):
        nc.scalar.activation(
            rT_sb[:, cg, :], rT_ps[:, cg, :], AFT.Relu,
            bias=shfr_sb[:, cg:cg + 1], scale=sclr_sb[:, cg:cg + 1],
        )
    o_ps = ps_l.tile([P, C], F32, tag="ps_l")
    for cg in range(2):
        nc.tensor.matmul(
            out=o_ps[:],
            lhsT=rT_sb[:, cg, :],
            rhs=rw2_sb[:, cg, :],
            start=(cg == 0), stop=(cg == 1),
        )
    out_sb = pool.tile([P, C], F32, tag="out_sb")
    nc.vector.tensor_tensor(
        out=out_sb[:],
        in0=o_ps[:],
        in1=rb_sb[:],
        op=AOP.add)
    nc.sync.dma_start(out=out_ext[base + j * P:base + (j + 1) * P, :],
                      in_=out_sb[:])


def prep_weights(inputs):
    """Host-side folding of BN/bias into matmul weights."""
    g1, b1, m1, v1 = [np.asarray(inputs["w_bn1"][i], np.float64)
                      for i in range(4)]
    scale1 = g1 / np.sqrt(v1 + EPS)
    mean_eff = m1 - (np.asarray(inputs["bk"], np.float64)
                     - np.asarray(inputs["bq"], np.float64)
                     + np.asarray(inputs["p_b2"], np.float64)
                     + np.asarray(inputs["v_b2"], np.float64))
    shift1 = b1 - scale1 * mean_eff

    wq_s = (scale1[:, None] * inputs["Wq"]).T  # [cin, cout]
    wqn_t = np.stack([_bf(-wq_s[0:P]), _bf(-wq_s[P:2 * P])])
    wk_s = (scale1[:, None] * inputs["Wk"]).T
    wv = np.asarray(inputs["Wv"], np.float64).T
    wkv = np.concatenate([wk_s, wv], axis=1)  # [256, 512]
    wkv_t = np.stack([_bf(wkv[0:P]), _bf(wkv[P:2 * P])])

    gp, bp, mp, vp = [np.asarray(inputs["p_bn"][i], np.float64)
                      for i in range(4)]
    scale_p = gp / np.sqrt(vp + EPS)
    A1 = scale_p[:, None] * inputs["p_w1"]
    c1 = bp - scale_p * (mp - inputs["p_b1"])
    a1_t = np.zeros((64, 8, 4), np.float32)
    for u in range(8):
        a1_t[8 * u:8 * u + 3, u, :3] = A1.T

    gv, bv_, mv, vv = [np.asarray(inputs["v_bn"][i], np.float64)
                      for i in range(4)]
    scale_v = (gv / np.sqrt(vv + EPS))[0]
    sv = scale_v * inputs["v_w1"][0, 0]
    bvp = scale_v * (inputs["v_b1"][0] - mv[0]) + bv_[0]

    # w2w5[8u+k, u, cg]: rows 0-2 p_w2.T*scale1, row 3 v_w2*scale1,
    # row 4 shift1; banded per within-group index u, zero elsewhere
    w2w5 = np.zeros((32, 4, 2, P), np.float64)
    pw2s = np.asarray(inputs["p_w2"], np.float64).T * scale1[None, :]
    vw2s = np.asarray(inputs["v_w2"], np.float64)[:, 0] * scale1
    for cg in range(2):
        sl = slice(cg * P, (cg + 1) * P)
        for u in range(4):
            w2w5[8 * u + 0:8 * u + 3, u, cg] = pw2s[:, sl]
            w2w5[8 * u + 3, u, cg] = vw2s[sl]
            w2w5[8 * u + 4, u, cg] = shift1[sl]

    # w2v_r: d-major packed rows for P4: [g*CS*4] with layout (g, cs, d)
    w2v = np.zeros((4, C), np.float64)
    w2v[0:3] = np.asarray(inputs["p_w2"], np.float64).T
    w2v[3] = np.asarray(inputs["v_w2"], np.float64)[:, 0]
    w2v_r = np.ascontiguousarray(
        w2v.T.reshape(S, CS, 4).reshape(1, -1))   # [(g cs d)]

    g2, b2, m2, v2 = [np.asarray(inputs["w_bn2"][i], np.float64)
                      for i in range(4)]
    scale2 = g2 / np.sqrt(v2 + EPS)
    shift2 = b2 - scale2 * (m2 - inputs["w_b1"])
    w1s = (scale2[:, None] * inputs["w_w1"]).T  # [256, 32]
    w1_t = np.stack([_bf(w1s[0:P]), _bf(w1s[P:2 * P])])
    w2_t = _bf(np.asarray(inputs["w_w2"]).T)

    gr, br, mr, vr = [np.asarray(inputs["r_bn"][i], np.float64)
                      for i in range(4)]
    scale_r = gr / np.sqrt(vr + EPS)
    mean_r = mr - (np.asarray(inputs["bv"], np.float64)
                   + inputs["p_b2"] + inputs["v_b2"])
    shift_r = br - scale_r * mean_r
    rw2s = np.asarray(inputs["r_w"]).T
    rw2 = np.stack([_bf(rw2s[0:P]), _bf(rw2s[P:2 * P])])

    return {
        "wqn_t": wqn_t,
        "wkv_t": wkv_t,
        "w2w5": _bf(w2w5),
        "w2v_r": _bf(w2v_r),
        "w1_t": w1_t,
        "w2_t": w2_t,
        "rw2": rw2,
        "a1_t": _f32(a1_t),
        "c1_c": _f32(np.pad(np.asarray(c1, np.float64), (0, 1))[None, :]),
        "svbv_c": _f32(np.array([[sv, bvp]])),
        "sh2_c": _f32(np.asarray(shift2)[:, None]),
        "sclr_c": _f32(np.asarray(scale_r).reshape(2, P).T),
        "shfr_c": _f32(np.asarray(shift_r).reshape(2, P).T),
        "rb_c": _f32(np.asarray(inputs["r_b"])[None, :]),
    }


def prep_inputs(inputs, n_total, use_collective=False):
    """Build the per-core input maps."""
    npc = n_total // NCORES
    wd = prep_weights(inputs)
    feats_bf = _bf(inputs["feats"])                    # [N, C]
    featsT = np.ascontiguousarray(
        feats_bf.T.reshape(2, P, n_total))             # [2, 128, N]
    xyz4 = np.zeros((n_total, 4), np.float32)
    xyz4[:, :3] = inputs["xyz"]
    velw = np.ascontiguousarray(
        _f32(inputs["velocities"]).reshape(P, n_total // P))

    in_maps = []
    for c in range(NCORES):
        sl = slice(c * npc, (c + 1) * npc)
        if use_collective:
            ftb = np.ascontiguousarray(featsT[:, :, sl])
            xyzb = np.ascontiguousarray(xyz4[sl])
            velb = np.ascontiguousarray(
                _f32(inputs["velocities"][sl]).reshape(P, npc // P))
        else:
            ftb, xyzb, velb = featsT, xyz4, velw
        m = {
            "featsT_b": ftb,
            "xyz4_b": xyzb,
            "vel_b": velb,
            "featsT_own": np.ascontiguousarray(featsT[:, :, sl]),
            "featsb_sh": np.ascontiguousarray(feats_bf[sl]),
            "xyz4_sh": np.ascontiguousarray(xyz4[sl]),
            "idx_sh": np.ascontiguousarray(inputs["idx"][sl], np.int32),
            "idxv_sh": np.ascontiguousarray(inputs["idx_v"][sl], np.int32),
            "idxvf_sh": wrap_fat_idx(inputs["idx_v"][sl]),
            "voh_sh": onehot64(inputs["idx_v"][sl]),
        }
        m.update(wd)
        in_maps.append(m)
    return in_maps


def wrap_fat_idx(idxv_shard):
    """Per-tile wrapped int16 layout of idx_v//64 for dma_gather."""
    npc_l = idxv_shard.shape[0]
    nt_l = npc_l // P
    ne = NS * P
    fat = (np.asarray(idxv_shard, np.int64) // 64).astype(np.int16)
    out = np.empty((nt_l, P, ne // 16), np.int16)
    for t in range(nt_l):
        flat = fat[t * P:(t + 1) * P].T.ravel()    # e = s*128 + p
        wrap = flat.reshape(ne // 16, 16).T         # [16, ne//16]
        out[t] = np.tile(wrap, (8, 1))
    return np.ascontiguousarray(out)


def onehot64(idxv_shard):
    """bf16 one-hot of idx_v % 64, [npc, NS*64]."""
    m = (np.asarray(idxv_shard, np.int64) % 64)
    npc_l = m.shape[0]
    oh = np.zeros((npc_l, NS, 64), np.float32)
    np.put_along_axis(oh, m[:, :, None], 1.0, axis=2)
    return _bf(oh.reshape(npc_l, NS * 64))


_PROGRAM_CACHE = {}


def get_program(n_total, use_collective=False):
    key = (n_total, use_collective)
    if key not in _PROGRAM_CACHE:
        _PROGRAM_CACHE[key] = build_program(n_total, use_collective)
    return _PROGRAM_CACHE[key]


def _pjrt_pieces(nc):
    import concourse.mybir as mb

    partition_name = (nc.partition_id_tensor.name
                      if nc.partition_id_tensor else None)
    in_names, out_names, out_avals, zero_outs = [], [], [], []
    import jax

    for alloc in nc.m.functions[0].allocations:
        if not isinstance(alloc, mb.MemoryLocationSet):
            continue
        name = alloc.memorylocations[0].name
        if alloc.kind == "ExternalInput":
            if name != partition_name:
                in_names.append(name)
        elif alloc.kind == "ExternalOutput":
            shape = tuple(alloc.tensor_shape)
            dtype = mb.dt.np(alloc.dtype)
            out_names.append(name)
            out_avals.append(jax.core.ShapedArray(shape, dtype))
            zero_outs.append(np.zeros(shape, dtype))
    return partition_name, in_names, out_names, out_avals, zero_outs


def run_timed(inputs, n_total, n_iter=8, use_collective=False):
    """Run on the 8 cores; returns (out, per-exec ns from chained timing)."""
    import time as _time

    import jax
    from jax.sharding import Mesh, PartitionSpec
    from jax.experimental.shard_map import shard_map

    from concourse import bass2jax
    from concourse.bass2jax import _bass_exec_p, partition_id_tensor

    inputs = {k: np.asarray(v) for k, v in inputs.items()}
    nc = get_program(n_total, use_collective)
    in_maps = prep_inputs(inputs, n_total, use_collective)
    bass2jax.install_neuronx_cc_hook()

    (partition_name, in_names, out_names, out_avals,
     zero_outs) = _pjrt_pieces(nc)
    n_params = len(in_names)
    n_outs = len(out_names)
    all_in_names = in_names + out_names
    if partition_name is not None:
        all_in_names = all_in_names + [partition_name]

    def body_n(n_chain):
        def _body(*args):
            ins = list(args[:n_params])
            zouts = list(args[n_params:n_params + n_outs])
            outs = None
            for _ in range(n_chain):
                ops = ins + zouts
                if partition_name is not None:
                    ops = ops + [partition_id_tensor()]
                outs = list(_bass_exec_p.bind(
                    *ops,
                    out_avals=tuple(out_avals),
                    in_names=tuple(all_in_names),
                    out_names=tuple(out_names),
                    lowering_input_output_aliases=(),
                    sim_require_finite=True,
                    sim_require_nnan=True,
                    nc=nc,
                ))
            return tuple(outs)
        return _body

    devices = jax.devices()[:NCORES]
    mesh = Mesh(np.asarray(devices), ("core",))
    in_specs = (PartitionSpec("core"),) * (n_params + n_outs)
    out_specs = (PartitionSpec("core"),) * n_outs
    per_core = [[np.asarray(m[name]) for name in in_names]
                for m in in_maps]
    concat_in = [np.concatenate([per_core[c][i] for c in range(NCORES)],
                                axis=0) for i in range(n_params)]
    concat_zero = [np.concatenate([z] * NCORES, axis=0) for z in zero_outs]
    args = [jax.device_put(a) for a in concat_in + concat_zero]

    # correctness run (single execution)
    fn1 = jax.jit(shard_map(body_n(1), mesh=mesh, in_specs=in_specs,
                            out_specs=out_specs, check_rep=False),
                  keep_unused=True)
    res = jax.block_until_ready(fn1(*args))
    out_full = np.asarray(res[out_names.index("out")])

    exec_ns = None
    if n_iter > 1:
        fnN = jax.jit(shard_map(body_n(n_iter), mesh=mesh,
                                in_specs=in_specs, out_specs=out_specs,
                                check_rep=False), keep_unused=True)
        r = jax.block_until_ready(fnN(*args))   # warmup/compile
        tN = None
        for _ in range(3):
            t1 = _time.perf_counter()
            r = jax.block_until_ready(fnN(*args))
            dt = _time.perf_counter() - t1
            tN = dt if tN is None else min(tN, dt)
        t1w = None
        for _ in range(3):
            t1 = _time.perf_counter()
            jax.block_until_ready(fn1(*args))
            dt = _time.perf_counter() - t1
            t1w = dt if t1w is None else min(t1w, dt)
        per = (tN - t1w) / (n_iter - 1)
        exec_ns = int(max(per, tN / n_iter) * 1e9) if per <= 0 else int(per * 1e9)
    return out_full, exec_ns


def run_timed2(inputs, n_total, n_iter=8):
    """Two-NEFF pipeline: prog1 builds the kv/vh tables (phase VB), prog2
    runs the main pass (phase D) reading the tables as device-resident
    arrays.  Avoids the single-program cross-phase scheduling pathology."""
    import time as _time

    import jax
    from jax.sharding import Mesh, PartitionSpec
    from jax.experimental.shard_map import shard_map

    from concourse import bass2jax
    from concourse.bass2jax import _bass_exec_p, partition_id_tensor

    inputs = {k: np.asarray(v) for k, v in inputs.items()}
    key = (n_total, "two")
    if key not in _PROGRAM_CACHE:
        _PROGRAM_CACHE[key] = (
            build_program(n_total, phases="VB", table_io="out"),
            build_program(n_total, phases="D", table_io="in"),
        )
    nc1, nc2 = _PROGRAM_CACHE[key]
    in_maps = prep_inputs(inputs, n_total, use_collective=False)
    bass2jax.install_neuronx_cc_hook()

    p1 = _pjrt_pieces(nc1)
    p2 = _pjrt_pieces(nc2)
    (pn1, in1, out1, av1, z1) = p1
    (pn2, in2, out2, av2, z2) = p2
    # prog2 inputs that come from prog1 outputs
    from_prog1 = [nm for nm in in2 if nm in out1]
    in2_host = [nm for nm in in2 if nm not in out1]

    host_names = list(dict.fromkeys(in1 + in2_host))
    n_host = len(host_names)

    def body_n(n_chain):
        def _body(*args):
            host = dict(zip(host_names, args[:n_host]))
            z1s = list(args[n_host:n_host + len(out1)])
            z2s = list(args[n_host + len(out1):n_host + len(out1) + len(out2)])
            outs2 = None
            for _ in range(n_chain):
                ops1 = [host[nm] for nm in in1] + z1s
                if pn1 is not None:
                    ops1 = ops1 + [partition_id_tensor()]
                outs1 = list(_bass_exec_p.bind(
                    *ops1, out_avals=tuple(av1),
                    in_names=tuple(in1 + out1 + ([pn1] if pn1 else [])),
                    out_names=tuple(out1),
                    lowering_input_output_aliases=(),
                    sim_require_finite=True, sim_require_nnan=True, nc=nc1,
                ))
                o1 = dict(zip(out1, outs1))
                ops2 = [o1[nm] if nm in o1 else host[nm] for nm in in2] + z2s
                if pn2 is not None:
                    ops2 = ops2 + [partition_id_tensor()]
                outs2 = list(_bass_exec_p.bind(
                    *ops2, out_avals=tuple(av2),
                    in_names=tuple(in2 + out2 + ([pn2] if pn2 else [])),
                    out_names=tuple(out2),
                    lowering_input_output_aliases=(),
                    sim_require_finite=True, sim_require_nnan=True, nc=nc2,
                ))
            return tuple(outs2)
        return _body

    devices = jax.devices()[:NCORES]
    mesh = Mesh(np.asarray(devices), ("core",))
    n_args = n_host + len(out1) + len(out2)
    in_specs = (PartitionSpec("core"),) * n_args
    out_specs = (PartitionSpec("core"),) * len(out2)
    concat_host = [
        np.concatenate([np.asarray(in_maps[c][nm]) for c in range(NCORES)],
                       axis=0) for nm in host_names]
    concat_z1 = [np.concatenate([z] * NCORES, axis=0) for z in z1]
    concat_z2 = [np.concatenate([z] * NCORES, axis=0) for z in z2]
    args = [jax.device_put(a) for a in concat_host + concat_z1 + concat_z2]

    fn1 = jax.jit(shard_map(body_n(1), mesh=mesh, in_specs=in_specs,
                            out_specs=out_specs, check_rep=False),
                  keep_unused=True)
    res = jax.block_until_ready(fn1(*args))
    out_full = np.asarray(res[out2.index("out")])

    exec_ns = None
    if n_iter > 1:
        fnN = jax.jit(shard_map(body_n(n_iter), mesh=mesh,
                                in_specs=in_specs, out_specs=out_specs,
                                check_rep=False), keep_unused=True)
        jax.block_until_ready(fnN(*args))
        tN = None
        for _ in range(3):
            t1 = _time.perf_counter()
            jax.block_until_ready(fnN(*args))
            dt = _time.perf_counter() - t1
            tN = dt if tN is None else min(tN, dt)
        t1w = None
        for _ in range(3):
            t1 = _time.perf_counter()
            jax.block_until_ready(fn1(*args))
            dt = _time.perf_counter() - t1
            t1w = dt if t1w is None else min(t1w, dt)
        per = (tN - t1w) / (n_iter - 1)
        exec_ns = int(max(per, tN / n_iter) * 1e9) if per <= 0 else int(per * 1e9)
    return out_full, exec_ns


def kernel(**inputs):
    inputs = {k: np.asarray(v) for k, v in inputs.items()}
    n_total = inputs["feats"].shape[0]
    out, _ = run_timed2(inputs, n_total, n_iter=1)
    return np.ascontiguousarray(out, dtype=np.float32)

